# revision 1
# baseline (speedup 1.0000x reference)
"""EvenNet (even-order PPR GNN) Trainium2 kernel, 8-core SPMD.

Math: hidden = sum_{k=0..5} temp[k] * A_hat^{2k} @ MLP(x); out = log_softmax(hidden)
with A_hat = D^-1/2 A D^-1/2 (D = out-degree from src indices).

Reformulation: iterate in y-space, y = D^-1/2 x.  One propagation x <- A_hat x
becomes y <- D^-1 * (A y) where (A y)[d] = sum_{e: dst=d} y[src_e] — a pure
unweighted gather + segment-sum (no per-edge weights).  Per-node scalings are
fused into the PSUM->SBUF copies.

Distribution: nodes sharded contiguously across 8 cores; edges routed to the
dst-owner core.  Per step each core:
  1. dma_gathers its edges' source rows (bf16, 2 nodes packed per 256B row)
     from a replicated y-table in DRAM,
  2. segment-sums them on the TensorEngine: per 128-edge block, a one-hot
     [edges x 128] matrix (built by DVE compare-vs-iota from precomputed dst
     offsets) is the matmul stationary; PSUM accumulates each 128-dst window,
  3. rescales windows into the bf16 y shard (ScalarE, per-partition scale),
  4. AllGathers the shard to rebuild the table.
dma_scatter_add is NOT used: its HBM read-modify-write loses colliding
updates on real hardware.

Edges are blocked by (window, table-half, src-parity): the gather reads one
table half per chunk (int16 index limit), and src parity selects which half
of the gathered 256B pair-row feeds the matmul.  Blocks are padded to 128
edges with dstoff=-1 dummies (one-hot row = 0 -> no contribution).
"""

import dataclasses
import math
import numpy as np
import ml_dtypes

from concourse import bacc, bass, mybir, tile
from concourse.bass_utils import run_bass_kernel_spmd
from concourse.masks import make_identity

F32 = mybir.dt.float32
BF16 = mybir.dt.bfloat16
I16 = mybir.dt.int16
AF = mybir.ActivationFunctionType
ALU = mybir.AluOpType
NPBF16 = ml_dtypes.bfloat16


@dataclasses.dataclass
class Cfg:
    n_cores: int = 8
    n: int = 100000          # real nodes
    cin: int = 500           # input channels
    hid: int = 256           # MLP hidden
    cout: int = 50           # classes
    k_half: int = 5          # outer iterations (2 props each)
    chunk_blocks: int = 8    # 128-edge blocks per dma_gather (<=1024 idxs: ring limit)
    sw: int = 4              # windows per super-window (PSUM banks / 2)
    f: int = 64              # padded feature dim

    @property
    def nsh(self):
        return int(math.ceil(self.n / self.n_cores / 128) * 128)

    @property
    def npad(self):
        return self.nsh * self.n_cores

    @property
    def nt(self):
        return self.nsh // 128

    @property
    def cpad(self):
        return int(math.ceil(self.cin / 128) * 128)

    @property
    def pairs(self):
        return self.npad // 2

    @property
    def n_halves(self):      # int16 gather index limit
        return int(math.ceil(self.pairs / 32640))

    @property
    def ph(self):            # pair rows per table half
        return int(math.ceil(self.pairs / self.n_halves))


CFG = Cfg()


@dataclasses.dataclass
class Plan:
    blocks: list            # (w, half, parity, first_of_w, last_of_w)
    chunks: list            # (half, b0, nb)
    nblk: int
    chunk_plan_repr: tuple  # hashable summary


def _wrap16(arr):
    """[L] -> [128, L/16]: element i at [i%16, i//16], replicated across the
    8 GPSIMD core partition groups."""
    L = arr.shape[0]
    assert L % 16 == 0
    w = np.ascontiguousarray(arr.reshape(L // 16, 16).T)
    return np.tile(w, (8, 1))


def _tile_major(v, cfg):
    """[nsh] -> [128, nt] with node t*128+p at [p, t]."""
    return np.ascontiguousarray(v.reshape(cfg.nt, 128).T)


# --------------------------------------------------------------------------
# host-side preprocessing
# --------------------------------------------------------------------------

def preprocess(x, edge_index, W1, b1, W2, b2, temp, cfg):
    n, NT = cfg.n, cfg.nt
    NH = cfg.n_halves
    src = np.asarray(edge_index[0]).astype(np.int64)
    dst = np.asarray(edge_index[1]).astype(np.int64)
    x = np.asarray(x, dtype=np.float32)
    W1 = np.asarray(W1, dtype=np.float32)
    b1 = np.asarray(b1, dtype=np.float32)
    W2 = np.asarray(W2, dtype=np.float32)
    b2 = np.asarray(b2, dtype=np.float32)
    temp = np.asarray(temp, dtype=np.float32)

    deg = np.bincount(src, minlength=n).astype(np.float32)
    dinv = np.where(deg > 0, 1.0 / np.sqrt(np.maximum(deg, 1e-12)), 0.0).astype(np.float32)
    dinv2 = (dinv * dinv).astype(np.float32)
    temps = [float(t) for t in temp]

    # ---- route & group edges:  (core) -> sort by (w, half, parity, src) ----
    owner = dst // cfg.nsh
    ngrp = NT * NH * 2
    counts = np.zeros((cfg.n_cores, ngrp), dtype=np.int64)
    per_core = []
    for c in range(cfg.n_cores):
        m = owner == c
        s_c = src[m]
        dl = dst[m] - c * cfg.nsh
        w = dl // 128
        half = (s_c // 2) // cfg.ph
        parity = s_c % 2
        gid = (w * NH + half) * 2 + parity
        o = np.lexsort((s_c, gid))
        s_c, dl, gid = s_c[o], dl[o], gid[o]
        counts[c] = np.bincount(gid, minlength=ngrp)
        per_core.append((s_c, dl, gid))

    nmax = counts.max(axis=0)
    nblocks = np.ceil(nmax / 128).astype(np.int64)          # per group
    # every window needs at least one block (else its PSUM is never produced)
    wblk = nblocks.reshape(NT, NH * 2).sum(axis=1)
    for w in np.where(wblk == 0)[0]:
        nblocks[w * NH * 2] = 1

    # ---- block & chunk plan (shared across cores) ----
    blocks = []
    chunks = []
    first_seen = {}
    for W0 in range(0, NT, cfg.sw):
        ws = range(W0, min(W0 + cfg.sw, NT))
        for h in range(NH):
            per_w = []
            for w in ws:
                lst = []
                for p in (0, 1):
                    lst += [(w, h, p)] * int(nblocks[(w * NH + h) * 2 + p])
                per_w.append(lst)
            # round-robin across windows: consecutive matmuls hit different
            # PSUM banks so accumulation chains pipeline
            run = []
            mi = 0
            while any(per_w):
                for lst in per_w:
                    if lst:
                        run.append(lst.pop(0))
            for i in range(0, len(run), cfg.chunk_blocks):
                sub = run[i:i + cfg.chunk_blocks]
                chunks.append((h, len(blocks) + i, len(sub)))
            blocks += run
    # first/last flags
    last_idx = {}
    for i, (w, h, p) in enumerate(blocks):
        if w not in first_seen:
            first_seen[w] = i
        last_idx[w] = i
    blocks = [(w, h, p, i == first_seen[w], i == last_idx[w])
              for i, (w, h, p) in enumerate(blocks)]
    nblk = len(blocks)
    tot = nblk * 128

    # per-group ordered list of its block stream indices (may be interleaved)
    gblocks = {}
    for i, (w, h, p, _, _) in enumerate(blocks):
        gblocks.setdefault((w * NH + h) * 2 + p, []).append(i)

    # ---- per-core index arrays ----
    gidx_maps, doff_maps = [], []
    for c in range(cfg.n_cores):
        s_c, dl, gid = per_core[c]
        g_arr = np.zeros(tot, dtype=np.int16)
        d_arr = np.full(tot, -1.0, dtype=np.float32)
        gb = np.concatenate([[0], np.cumsum(counts[c])])
        for g in range(ngrp):
            cnt = int(counts[c][g])
            if cnt == 0:
                continue
            bl = np.asarray(gblocks[g], dtype=np.int64)
            j = np.arange(cnt)
            pos = bl[j // 128] * 128 + (j % 128)
            sl = slice(gb[g], gb[g + 1])
            g_arr[pos] = ((s_c[sl] // 2) - (s_c[sl] // 2 // cfg.ph) * cfg.ph
                          ).astype(np.int16)
            d_arr[pos] = (dl[sl] % 128).astype(np.float32)
        gidx_maps.append(_wrap16(g_arr))
        # dstoff: [128, nblk] column per block
        doff_maps.append(np.ascontiguousarray(
            d_arr.reshape(nblk, 128).T).astype(NPBF16))

    # ---- dense per-core inputs ----
    use_b1 = bool(np.any(b1))
    use_b2 = bool(np.any(b2))
    W1p = np.zeros((cfg.cpad, cfg.hid), dtype=np.float32)
    W1p[: cfg.cin] = W1
    W2p = np.zeros((cfg.hid, cfg.f), dtype=np.float32)
    W2p[:, : cfg.cout] = W2
    iota = np.tile(np.arange(128, dtype=np.float32)[None, :], (128, 1)).astype(NPBF16)
    in_maps = []
    for c in range(cfg.n_cores):
        lo = c * cfg.nsh
        real = max(0, min(cfg.nsh, n - lo))
        xs = np.zeros((cfg.nsh, cfg.cpad), dtype=np.float32)
        if real > 0:
            xs[:real, : cfg.cin] = x[lo:lo + real]
        sl = slice(lo, lo + real)
        dv = np.zeros(cfg.nsh, np.float32); dv[:real] = dinv[sl]
        dv2 = np.zeros(cfg.nsh, np.float32); dv2[:real] = dinv2[sl]
        tdv = np.zeros((128, cfg.k_half * NT), np.float32)
        for m in range(1, cfg.k_half + 1):
            tdv[:, (m - 1) * NT:m * NT] = _tile_major(
                (temps[m] * dv).astype(np.float32), cfg)
        m = {
            "xs": xs,
            "w1": W1p,
            "w2": W2p,
            "gidx": gidx_maps[c],
            "dstoff": doff_maps[c],
            "iota": iota,
            "dinv_t": _tile_major(dv, cfg),
            "dinv2_t": _tile_major(dv2, cfg),
            "tdinv": tdv,
        }
        if use_b1:
            m["b1"] = b1.reshape(1, cfg.hid).astype(np.float32)
        if use_b2:
            b2p = np.zeros((1, cfg.f), np.float32)
            b2p[0, : cfg.cout] = b2
            m["b2"] = b2p
        in_maps.append(m)

    plan = Plan(blocks=blocks, chunks=chunks, nblk=nblk,
                chunk_plan_repr=tuple(chunks))
    return in_maps, plan, temps, use_b1, use_b2


# --------------------------------------------------------------------------
# program builder
# --------------------------------------------------------------------------

def build_program(cfg, plan, temps, use_b1, use_b2):
    nc = bacc.Bacc("TRN2", target_bir_lowering=False, debug=False,
                   num_devices=cfg.n_cores, num_swdge_queues=4)

    NT, F, NSH = cfg.nt, cfg.f, cfg.nsh
    NCH, NHC = cfg.cpad // 128, cfg.hid // 128
    nblk = plan.nblk
    totc = nblk * 8      # int16 columns (128 edges/block / 16)
    CB = cfg.chunk_blocks

    xs_d = nc.declare_dram_parameter("xs", [NSH, cfg.cpad], F32, isOutput=False)
    w1_d = nc.declare_dram_parameter("w1", [cfg.cpad, cfg.hid], F32, isOutput=False)
    w2_d = nc.declare_dram_parameter("w2", [cfg.hid, F], F32, isOutput=False)
    gidx_d = nc.declare_dram_parameter("gidx", [128, totc], I16, isOutput=False)
    doff_d = nc.declare_dram_parameter("dstoff", [128, nblk], BF16, isOutput=False)
    iota_d = nc.declare_dram_parameter("iota", [128, 128], BF16, isOutput=False)
    dinv_d = nc.declare_dram_parameter("dinv_t", [128, NT], F32, isOutput=False)
    dinv2_d = nc.declare_dram_parameter("dinv2_t", [128, NT], F32, isOutput=False)
    tdinv_d = nc.declare_dram_parameter("tdinv", [128, cfg.k_half * NT], F32,
                                        isOutput=False)
    b1_d = nc.declare_dram_parameter("b1", [1, cfg.hid], F32, isOutput=False) if use_b1 else None
    b2_d = nc.declare_dram_parameter("b2", [1, F], F32, isOutput=False) if use_b2 else None
    out_d = nc.declare_dram_parameter("out", [NSH, cfg.cout], F32, isOutput=True)

    table = nc.dram_tensor("ytable", [cfg.npad, F], BF16, addr_space="Shared")
    bounce = nc.dram_tensor("ybounce", [NSH, F], BF16)
    tblv = table[:].rearrange("(a b) f -> a (b f)", b=2)    # [pairs, 128]

    n_steps = 2 * cfg.k_half
    rg = [list(range(cfg.n_cores))]

    with tile.TileContext(nc) as tc:
        with (
            tc.tile_pool(name="const", bufs=1) as constp,
            tc.tile_pool(name="persist", bufs=1) as persist,
        ):
            # ---- constants ----
            w1sb = constp.tile([128, NCH * cfg.hid], F32)
            for c in range(NCH):
                nc.sync.dma_start(w1sb[:, c * cfg.hid:(c + 1) * cfg.hid],
                                  w1_d[c * 128:(c + 1) * 128, :])
            w2sb = constp.tile([128, NHC * F], F32)
            for c in range(NHC):
                nc.sync.dma_start(w2sb[:, c * F:(c + 1) * F],
                                  w2_d[c * 128:(c + 1) * 128, :])
            dinv_t = constp.tile([128, NT], F32)
            nc.sync.dma_start(dinv_t[:], dinv_d[:])
            dinv2_t = constp.tile([128, NT], F32)
            nc.sync.dma_start(dinv2_t[:], dinv2_d[:])
            tdinv_t = constp.tile([128, cfg.k_half * NT], F32)
            nc.sync.dma_start(tdinv_t[:], tdinv_d[:])
            doff_sb = constp.tile([128, nblk], BF16)
            nc.sync.dma_start(doff_sb[:], doff_d[:])
            gidx_sb = constp.tile([128, nblk * 8], I16)
            nc.sync.dma_start(gidx_sb[:], gidx_d[:])
            iota_sb = constp.tile([128, 128], BF16)
            nc.sync.dma_start(iota_sb[:], iota_d[:])
            iota3 = iota_sb[:].rearrange("p (a f) -> p a f", a=1)
            if use_b1 or use_b2:
                ones1 = constp.tile([1, 128], F32)
                nc.vector.memset(ones1[:], 1.0)
            if use_b1:
                b1sb = constp.tile([1, cfg.hid], F32)
                nc.sync.dma_start(b1sb[:], b1_d[:])
            if use_b2:
                b2sb = constp.tile([1, F], F32)
                nc.sync.dma_start(b2sb[:], b2_d[:])

            hid_sb = persist.tile([128, NT * F], F32)
            ysb = persist.tile([128, NT * F], BF16)
            hid3 = hid_sb[:].rearrange("p (t f) -> p t f", f=F)
            y3 = ysb[:].rearrange("p (t f) -> p t f", f=F)

            # ---- MLP ----
            with (
                tc.tile_pool(name="xload", bufs=3) as xload,
                tc.tile_pool(name="xT", bufs=8) as xTp,
                tc.tile_pool(name="h1", bufs=2) as h1p,
                tc.tile_pool(name="h1T", bufs=4) as h1Tp,
                tc.tile_pool(name="psT", bufs=4, space="PSUM") as psT,
                tc.tile_pool(name="psH", bufs=2, space="PSUM") as psH,
                tc.tile_pool(name="psO", bufs=2, space="PSUM") as psO,
            ):
                ident = constp.tile([128, 128], F32)
                make_identity(nc, ident[:])
                for t in range(NT):
                    xt = xload.tile([128, cfg.cpad], F32)
                    nc.sync.dma_start(xt[:], xs_d[t * 128:(t + 1) * 128, :])
                    xTs = []
                    for c in range(NCH):
                        pt = psT.tile([128, 128], F32)
                        nc.tensor.transpose(pt[:], xt[:, c * 128:(c + 1) * 128],
                                            ident[:])
                        xTc = xTp.tile([128, 128], F32)
                        if c % 2 == 0:
                            nc.vector.tensor_copy(xTc[:], pt[:])
                        else:
                            nc.scalar.mul(xTc[:], pt[:], 1.0)
                        xTs.append(xTc)
                    h1ps = psH.tile([128, cfg.hid], F32)
                    for c in range(NCH):
                        nc.tensor.matmul(h1ps[:], lhsT=xTs[c][:],
                                         rhs=w1sb[:, c * cfg.hid:(c + 1) * cfg.hid],
                                         start=(c == 0),
                                         stop=(c == NCH - 1 and not use_b1))
                    if use_b1:
                        nc.tensor.matmul(h1ps[:], lhsT=ones1[:], rhs=b1sb[:],
                                         start=False, stop=True)
                    h1 = h1p.tile([128, cfg.hid], F32)
                    nc.scalar.activation(h1[:], h1ps[:], AF.Relu)
                    h1Ts = []
                    for c in range(NHC):
                        pt = psT.tile([128, 128], F32)
                        nc.tensor.transpose(pt[:], h1[:, c * 128:(c + 1) * 128],
                                            ident[:])
                        hTc = h1Tp.tile([128, 128], F32)
                        if c % 2 == 0:
                            nc.vector.tensor_copy(hTc[:], pt[:])
                        else:
                            nc.scalar.mul(hTc[:], pt[:], 1.0)
                        h1Ts.append(hTc)
                    hps = psO.tile([128, F], F32)
                    for c in range(NHC):
                        nc.tensor.matmul(hps[:], lhsT=h1Ts[c][:],
                                         rhs=w2sb[:, c * F:(c + 1) * F],
                                         start=(c == 0),
                                         stop=(c == NHC - 1 and not use_b2))
                    if use_b2:
                        nc.tensor.matmul(hps[:], lhsT=ones1[:], rhs=b2sb[:],
                                         start=False, stop=True)
                    nc.scalar.mul(hid_sb[:, t * F:(t + 1) * F], hps[:], temps[0])
                    nc.vector.tensor_scalar_mul(ysb[:, t * F:(t + 1) * F],
                                                hps[:], dinv_t[:, t:t + 1])

            bounce3 = bounce[:].rearrange("(t p) f -> p t f", p=128)

            def allgather():
                nc.sync.dma_start(bounce3, y3)
                nc.gpsimd.collective_compute(
                    "AllGather", ALU.bypass, replica_groups=rg,
                    ins=[bounce[:]], outs=[table[:]],
                )

            allgather()

            # ---- propagation steps ----
            with (
                tc.tile_pool(name="msg", bufs=10) as msgp,
                tc.tile_pool(name="oh", bufs=8) as ohp,
                tc.tile_pool(name="tw", bufs=4) as twp,
                tc.tile_pool(name="win", bufs=2 * cfg.sw, space="PSUM") as winp,
            ):
                gq = 0   # global Pool-DMA counter: keeps Tile's DMASW lane
                         # rotation (mod 8) consistent with queue_num (mod 4)
                for s in range(1, n_steps + 1):
                    psums = {}
                    for ci, (half, b0, nb) in enumerate(plan.chunks):
                        L = nb * 128
                        msg = msgp.tile([128, CB, 128], BF16, tag="msg")
                        hi = half * cfg.ph
                        tbl_half = tblv[hi:min(hi + cfg.ph, cfg.pairs), :]
                        nc.gpsimd.dma_gather(
                            msg[:, :nb, :], tbl_half,
                            gidx_sb[:, b0 * 8:b0 * 8 + L // 16], L, L, 128,
                            queue_num=gq % 4)
                        gq += 1
                        oh = ohp.tile([128, CB * 128], BF16, tag="oh")
                        oh3 = oh[:].rearrange("p (b f) -> p b f", f=128)
                        nc.vector.tensor_tensor(
                            oh3[:, :nb, :],
                            doff_sb[:, b0:b0 + nb].to_broadcast([128, nb, 128]),
                            iota3.to_broadcast([128, nb, 128]),
                            ALU.is_equal)
                        for j in range(nb):
                            w, h, p, first, last = plan.blocks[b0 + j]
                            if first:
                                psums[w] = winp.tile([128, F], F32, tag="win",
                                                     name=f"win_s{s}_w{w}")
                            nc.tensor.matmul(
                                psums[w][:], lhsT=oh3[:, j, :],
                                rhs=msg[:, j, p * 64:(p + 1) * 64],
                                start=first, stop=last)
                            if last:
                                ps = psums.pop(w)
                                nc.scalar.activation(
                                    y3[:, w, :], ps[:], AF.Copy,
                                    scale=dinv2_t[:, w:w + 1])
                                if s % 2 == 0:
                                    m = s // 2
                                    tw = twp.tile([128, F], F32, tag="tw")
                                    nc.scalar.activation(
                                        tw[:], ps[:], AF.Copy,
                                        scale=tdinv_t[:, (m - 1) * NT + w:
                                                      (m - 1) * NT + w + 1])
                                    nc.vector.tensor_add(
                                        hid3[:, w, :], hid3[:, w, :], tw[:])
                    if s < n_steps:
                        allgather()

                # ---- log_softmax ----
                with tc.tile_pool(name="soft", bufs=1) as softp:
                    CO = cfg.cout
                    hsl = hid3[:, :, :CO]
                    mx = softp.tile([128, NT], F32, tag="mx")
                    nc.vector.tensor_reduce(mx[:], hsl, mybir.AxisListType.X,
                                            ALU.max)
                    ex = softp.tile([128, NT * F], F32, tag="scratch")
                    ex3 = ex[:].rearrange("p (t f) -> p t f", f=F)[:, :, :CO]
                    nc.vector.tensor_tensor(
                        ex3, hsl, mx[:].to_broadcast([128, NT, CO]),
                        ALU.subtract)
                    nc.scalar.activation(ex3, ex3, AF.Exp)
                    sm = softp.tile([128, NT], F32, tag="sm")
                    nc.vector.tensor_reduce(sm[:], ex3, mybir.AxisListType.X,
                                            ALU.add)
                    ln = softp.tile([128, NT], F32, tag="ln")
                    nc.scalar.activation(ln[:], sm[:], AF.Ln)
                    ml = softp.tile([128, NT], F32, tag="ml")
                    nc.vector.tensor_add(ml[:], mx[:], ln[:])
                    ot = softp.tile([128, NT * CO], F32, tag="ot")
                    ot3 = ot[:].rearrange("p (t f) -> p t f", f=CO)
                    nc.vector.tensor_tensor(
                        ot3, hsl, ml[:].to_broadcast([128, NT, CO]),
                        ALU.subtract)
                    out3 = out_d[:].rearrange("(t p) f -> p t f", p=128)
                    nc.sync.dma_start(out3, ot3)

    nc.compile()
    return nc


# --------------------------------------------------------------------------
# entry point
# --------------------------------------------------------------------------

def kernel_with_results(x, edge_index, W1, b1, W2, b2, temp, trace=False):
    cfg = CFG
    in_maps, plan, temps, use_b1, use_b2 = preprocess(
        x, edge_index, W1, b1, W2, b2, temp, cfg)
    nc = build_program(cfg, plan, temps, use_b1, use_b2)
    res = run_bass_kernel_spmd(nc, in_maps, core_ids=list(range(cfg.n_cores)),
                               trace=trace)
    outs = [res.results[c]["out"] for c in range(cfg.n_cores)]
    full = np.concatenate(outs, axis=0)[: cfg.n]
    return full.astype(np.float32), res


def kernel(x, edge_index, W1, b1, W2, b2, temp):
    out, _ = kernel_with_results(x, edge_index, W1, b1, W2, b2, temp)
    return out



# revision 4
# speedup vs baseline: 1.1168x; 1.1168x over previous
"""EvenNet (even-order PPR GNN) Trainium2 kernel, 8-core SPMD.

Math: hidden = sum_{k=0..5} temp[k] * A_hat^{2k} @ MLP(x); out = log_softmax(hidden)
with A_hat = D^-1/2 A D^-1/2 (D = out-degree from src indices).

Reformulation: iterate in y-space, y = D^-1/2 x.  One propagation x <- A_hat x
becomes y <- D^-1 * (A y) where (A y)[d] = sum_{e: dst=d} y[src_e] — a pure
unweighted gather + segment-sum.  Per-node scalings fold into window flushes.

Distribution: nodes sharded contiguously across 8 cores; edges routed to the
dst-owner core.  The replicated y-table is split into two DRAM halves, each
interleaving every core's half-shard:
  tableA row c*6272 + l  = y of node c*12544 + l         (l < 6272)
  tableB row c*6272 + l' = y of node c*12544 + 6272 + l'
so pair indices stay < 25088 (int16 dma_gather limit) AND each half can be
AllGathered independently: AG_A fires mid-step (after local windows 0..48
flush), AG_B at step end.  The next step's A-pass gathers depend only on
AG_A, so the Q7 descriptor generator runs straight through the AG_B barrier.

Per step each core runs an A-pass then a B-pass over its 98 dst windows:
gather chunks (1024 edges, 4 SWDGE queues) read one table half; per 128-edge
block a one-hot [edges x 128] fp8 matrix (DVE compare vs iota from
precomputed dst offsets) scatter-sums gathered bf16 pair-rows on the
TensorEngine into the window PSUM.  Blocks mix both parities: edges are
sorted by parity inside each (window, half) group, so most blocks emit one
matmul; the straddling block emits two (one per parity, disjoint one-hot
columns).  A-pass partials stash to SBUF f32; B-pass adds them back and
applies the D^-1 / temp scalings.
"""

import dataclasses
import math
import numpy as np
import ml_dtypes

from concourse import bacc, bass, mybir, tile
from concourse.bass_utils import run_bass_kernel_spmd
from concourse.masks import make_identity

F32 = mybir.dt.float32
BF16 = mybir.dt.bfloat16
FP8 = mybir.dt.float8e4
I16 = mybir.dt.int16
AF = mybir.ActivationFunctionType
ALU = mybir.AluOpType
NPBF16 = ml_dtypes.bfloat16


@dataclasses.dataclass
class Cfg:
    n_cores: int = 8
    n: int = 100000          # real nodes
    cin: int = 500           # input channels
    hid: int = 256           # MLP hidden
    cout: int = 50           # classes
    k_half: int = 5          # outer iterations (2 props each)
    chunk_blocks: int = 8    # 128-edge blocks per dma_gather (1024 = ring cap)
    sw: int = 4              # windows per super-window (PSUM rotation)
    f: int = 64              # padded feature dim

    @property
    def nsh(self):           # nodes per core (padded)
        return int(math.ceil(self.n / self.n_cores / 128) * 128)

    @property
    def nt(self):            # windows per core
        return self.nsh // 128

    @property
    def hw_(self):           # windows in half A
        return (self.nt + 1) // 2

    @property
    def hn(self):            # nodes per half-shard
        return self.hw_ * 128

    @property
    def cpad(self):
        return int(math.ceil(self.cin / 128) * 128)

    @property
    def tab_rows(self):      # rows per table half
        return self.hn * self.n_cores

    @property
    def tab_pairs(self):
        return self.tab_rows // 2


CFG = Cfg()


@dataclasses.dataclass
class Plan:
    blocks: list     # (w, start, stop, segs) per stream block; segs=[(par, col)]
    chunks: list     # (h, b0, nb, col0, ncols)
    nblk: int
    ncols: int
    ag1_chunk: int   # chunk index after which windows 0..hw_-1 have flushed


def _wrap16(arr):
    L = arr.shape[0]
    assert L % 16 == 0
    w = np.ascontiguousarray(arr.reshape(L // 16, 16).T)
    return np.tile(w, (8, 1))


def _tile_major(v, cfg):
    return np.ascontiguousarray(v.reshape(cfg.nt, 128).T)


# --------------------------------------------------------------------------
# host-side preprocessing
# --------------------------------------------------------------------------

def preprocess(x, edge_index, W1, b1, W2, b2, temp, cfg):
    n, NT, HW, HN = cfg.n, cfg.nt, cfg.hw_, cfg.hn
    NSH = cfg.nsh
    src = np.asarray(edge_index[0]).astype(np.int64)
    dst = np.asarray(edge_index[1]).astype(np.int64)
    x = np.asarray(x, dtype=np.float32)
    W1 = np.asarray(W1, dtype=np.float32)
    W2 = np.asarray(W2, dtype=np.float32)
    b1 = np.asarray(b1, dtype=np.float32)
    b2 = np.asarray(b2, dtype=np.float32)
    temp = np.asarray(temp, dtype=np.float32)
    assert not np.any(b1) and not np.any(b2), "bias path removed"

    deg = np.bincount(src, minlength=n).astype(np.float32)
    dinv = np.where(deg > 0, 1.0 / np.sqrt(np.maximum(deg, 1e-12)), 0.0).astype(np.float32)
    dinv2 = (dinv * dinv).astype(np.float32)
    temps = [float(t) for t in temp]

    # ---- per-edge routing ----
    c_s = src // NSH
    l_s = src % NSH
    half = l_s // HN                       # which table
    row = c_s * HN + (l_s % HN)            # row within table half
    pair = row // 2
    par = row % 2

    owner = dst // NSH
    ngrp = NT * 2                          # (w, half)
    counts = np.zeros((cfg.n_cores, ngrp, 2), dtype=np.int64)
    per_core = []
    for c in range(cfg.n_cores):
        m = owner == c
        dl = dst[m] - c * NSH
        w = dl // 128
        gid = w * 2 + half[m]
        o = np.lexsort((pair[m], par[m], gid))
        per_core.append((pair[m][o], (dl % 128)[o], par[m][o], gid[o]))
        cnt2 = np.zeros((ngrp, 2), dtype=np.int64)
        np.add.at(cnt2, (gid[o], par[m][o]), 1)
        counts[c] = cnt2

    tot_cnt = counts.sum(axis=2)           # [cores, ngrp]
    nblocks = np.maximum(np.ceil(tot_cnt.max(axis=0) / 128).astype(np.int64), 1)

    # ---- shared block stream: A-pass then B-pass, sw-rotated windows ----
    blocks = []            # (w, gid) per stream slot
    chunk_bounds = []      # (h, start_block, nb) chunks
    for h in (0, 1):
        pass_start = len(blocks)
        for W0 in range(0, NT, cfg.sw):
            per_w = []
            for w in range(W0, min(W0 + cfg.sw, NT)):
                per_w.append([(w, w * 2 + h)] * int(nblocks[w * 2 + h]))
            while any(per_w):
                for lst in per_w:
                    if lst:
                        blocks.append(lst.pop(0))
        for i in range(pass_start, len(blocks), cfg.chunk_blocks):
            nb = min(cfg.chunk_blocks, len(blocks) - i)
            chunk_bounds.append((h, i, nb))
    nblk = len(blocks)

    # first/last per (w, h) pass segment
    first_seen, last_idx = {}, {}
    for i, (w, gid) in enumerate(blocks):
        if gid not in first_seen:
            first_seen[gid] = i
        last_idx[gid] = i

    # ordered stream positions of each group's blocks (interleaved by the
    # super-window round-robin)
    gblocks = {}
    for i, (w, gid) in enumerate(blocks):
        gblocks.setdefault(gid, []).append(i)

    # ---- segments (parity runs) per block ----
    # Per core, group gid's slots: par0 edges at [0, cnt0), par1 at
    # [cnt0, cnt0+cnt1).  Shared flags OR over cores.
    cnt0 = counts[:, :, 0]                 # per core
    cnt1 = counts[:, :, 1]
    has = np.zeros((nblk, 2), dtype=bool)
    for g in range(ngrp):
        if g not in gblocks:
            continue
        bl = gblocks[g]
        for k, bi in enumerate(bl):
            lo, hi = k * 128, (k + 1) * 128
            for c in range(cfg.n_cores):
                c0 = int(cnt0[c, g])
                ctot = c0 + int(cnt1[c, g])
                if min(c0, hi) > lo:
                    has[bi, 0] = True
                if min(ctot, hi) > max(c0, lo):
                    has[bi, 1] = True
        if not has[bl].any():
            has[bl[0], 0] = True           # empty group: dummy even segment

    # column assignment (one one-hot column per (block, parity) segment)
    segs_per_block = []
    col = 0
    for i in range(nblk):
        segs = [(p, None) for p in (0, 1) if has[i, p]]
        segs = [(p, col + j) for j, (p, _) in enumerate(segs)]
        col += len(segs)
        segs_per_block.append(segs)
    ncols = col

    blocks_meta = []
    for i, (w, gid) in enumerate(blocks):
        blocks_meta.append((w, i == first_seen[gid], i == last_idx[gid],
                            segs_per_block[i]))

    # chunks with column ranges
    chunks = []
    for (h, b0, nb) in chunk_bounds:
        col0 = segs_per_block[b0][0][1]
        last = segs_per_block[b0 + nb - 1]
        colend = last[-1][1] + 1
        chunks.append((h, b0, nb, col0, colend - col0))

    # AG1 point: after the chunk containing the stop-B block of window HW-1
    wtarget = HW - 1
    stopB_blk = last_idx[wtarget * 2 + 1]
    ag1_chunk = next(ci for ci, (h, b0, nb, _, _) in enumerate(chunks)
                     if h == 1 and b0 <= stopB_blk < b0 + nb)

    # ---- per-core index / dstoff arrays ----
    tot = nblk * 128
    gidx_maps, doff_maps = [], []
    for c in range(cfg.n_cores):
        pr, dl, pa, gid = per_core[c]
        g_arr = np.zeros(tot, dtype=np.int16)
        d_arr = np.full((ncols, 128), -1.0, dtype=np.float32)
        gb = np.concatenate([[0], np.cumsum(tot_cnt[c])])
        for g in range(ngrp):
            cnt = int(tot_cnt[c, g])
            if cnt == 0:
                continue
            bl = np.asarray(gblocks[g], dtype=np.int64)
            sl = slice(gb[g], gb[g + 1])
            j = np.arange(cnt)
            blk = bl[j // 128]
            slot = j % 128
            pos = blk * 128 + slot
            g_arr[pos] = pr[sl].astype(np.int16)
            # dstoff goes to the column matching (block, parity)
            par_e = pa[sl]
            for b in np.unique(blk):
                mb = blk == b
                for (p, colid) in segs_per_block[b]:
                    mm = mb & (par_e == p)
                    d_arr[colid, slot[mm]] = dl[sl][mm]
        gidx_maps.append(_wrap16(g_arr))
        doff_maps.append(np.ascontiguousarray(d_arr.T).astype(NPBF16))

    # ---- dense per-core inputs (bf16 MLP) ----
    W1p = np.zeros((cfg.cpad, cfg.hid), dtype=NPBF16)
    W1p[: cfg.cin] = W1.astype(NPBF16)
    W2p = np.zeros((cfg.hid, cfg.f), dtype=NPBF16)
    W2p[:, : cfg.cout] = W2.astype(NPBF16)
    iota = np.tile(np.arange(128, dtype=np.float32)[None, :], (128, 1)).astype(NPBF16)
    in_maps = []
    for c in range(cfg.n_cores):
        lo = c * NSH
        real = max(0, min(NSH, n - lo))
        xs = np.zeros((NSH, cfg.cpad), dtype=NPBF16)
        if real > 0:
            xs[:real, : cfg.cin] = x[lo:lo + real].astype(NPBF16)
        dv = np.zeros(NSH, np.float32)
        dv[:real] = dinv[lo:lo + real]
        dv2 = np.zeros(NSH, np.float32)
        dv2[:real] = dinv2[lo:lo + real]
        tdv = np.zeros((128, cfg.k_half * NT), np.float32)
        for m in range(1, cfg.k_half + 1):
            tdv[:, (m - 1) * NT:m * NT] = _tile_major(
                (temps[m] * dv).astype(np.float32), cfg)
        in_maps.append({
            "xs": xs,
            "w1": W1p,
            "w2": W2p,
            "gidx": gidx_maps[c],
            "dstoff": doff_maps[c],
            "iota": iota,
            "dinv_t": _tile_major(dv, cfg),
            "dinv2_t": _tile_major(dv2, cfg),
            "tdinv": tdv,
        })

    plan = Plan(blocks=blocks_meta, chunks=chunks, nblk=nblk, ncols=ncols,
                ag1_chunk=ag1_chunk)
    return in_maps, plan, temps


# --------------------------------------------------------------------------
# program builder
# --------------------------------------------------------------------------

def build_program(cfg, plan, temps):
    nc = bacc.Bacc("TRN2", target_bir_lowering=False, debug=False,
                   num_devices=cfg.n_cores, num_swdge_queues=4)

    NT, F, NSH, HW, HN = cfg.nt, cfg.f, cfg.nsh, cfg.hw_, cfg.hn
    NCH, NHC = cfg.cpad // 128, cfg.hid // 128
    nblk, ncols = plan.nblk, plan.ncols
    CB = cfg.chunk_blocks
    MAXC = 2 * CB

    xs_d = nc.declare_dram_parameter("xs", [NSH, cfg.cpad], BF16, isOutput=False)
    w1_d = nc.declare_dram_parameter("w1", [cfg.cpad, cfg.hid], BF16, isOutput=False)
    w2_d = nc.declare_dram_parameter("w2", [cfg.hid, F], BF16, isOutput=False)
    gidx_d = nc.declare_dram_parameter("gidx", [128, nblk * 8], I16, isOutput=False)
    doff_d = nc.declare_dram_parameter("dstoff", [128, ncols], BF16, isOutput=False)
    iota_d = nc.declare_dram_parameter("iota", [128, 128], BF16, isOutput=False)
    dinv_d = nc.declare_dram_parameter("dinv_t", [128, NT], F32, isOutput=False)
    dinv2_d = nc.declare_dram_parameter("dinv2_t", [128, NT], F32, isOutput=False)
    tdinv_d = nc.declare_dram_parameter("tdinv", [128, cfg.k_half * NT], F32,
                                        isOutput=False)
    out_d = nc.declare_dram_parameter("out", [NSH, cfg.cout], F32, isOutput=True)

    tabA = nc.dram_tensor("ytabA", [cfg.tab_rows, F], BF16, addr_space="Shared")
    tabB = nc.dram_tensor("ytabB", [cfg.tab_rows, F], BF16, addr_space="Shared")
    bounceA = nc.dram_tensor("ybounceA", [HN, F], BF16)
    bounceB = nc.dram_tensor("ybounceB", [HN, F], BF16)
    tabA_v = tabA[:].rearrange("(a b) f -> a (b f)", b=2)   # [pairs, 128]
    tabB_v = tabB[:].rearrange("(a b) f -> a (b f)", b=2)

    n_steps = 2 * cfg.k_half
    rg = [list(range(cfg.n_cores))]

    with tile.TileContext(nc) as tc:
        with (
            tc.tile_pool(name="const", bufs=1) as constp,
            tc.tile_pool(name="persist", bufs=1) as persist,
        ):
            w1sb = constp.tile([128, NCH * cfg.hid], BF16)
            for c in range(NCH):
                nc.sync.dma_start(w1sb[:, c * cfg.hid:(c + 1) * cfg.hid],
                                  w1_d[c * 128:(c + 1) * 128, :])
            w2sb = constp.tile([128, NHC * F], BF16)
            for c in range(NHC):
                nc.sync.dma_start(w2sb[:, c * F:(c + 1) * F],
                                  w2_d[c * 128:(c + 1) * 128, :])
            dinv_t = constp.tile([128, NT], F32)
            nc.sync.dma_start(dinv_t[:], dinv_d[:])
            dinv2_t = constp.tile([128, NT], F32)
            nc.sync.dma_start(dinv2_t[:], dinv2_d[:])
            tdinv_t = constp.tile([128, cfg.k_half * NT], F32)
            nc.sync.dma_start(tdinv_t[:], tdinv_d[:])
            doff_sb = constp.tile([128, ncols], BF16)
            nc.sync.dma_start(doff_sb[:], doff_d[:])
            gidx_sb = constp.tile([128, nblk * 8], I16)
            nc.sync.dma_start(gidx_sb[:], gidx_d[:])
            iota_sb = constp.tile([128, 128], BF16)
            nc.sync.dma_start(iota_sb[:], iota_d[:])
            iota3 = iota_sb[:].rearrange("p (a f) -> p a f", a=1)

            hid_sb = persist.tile([128, NT * F], F32)
            ysb = persist.tile([128, NT * F], BF16)
            stash = persist.tile([128, NT * F], F32)
            hid3 = hid_sb[:].rearrange("p (t f) -> p t f", f=F)
            y3 = ysb[:].rearrange("p (t f) -> p t f", f=F)
            st3 = stash[:].rearrange("p (t f) -> p t f", f=F)

            bounceA3 = bounceA[:].rearrange("(t p) f -> p t f", p=128)
            bounceB3 = bounceB[:].rearrange("(t p) f -> p t f", p=128)

            def ag_a():
                nc.sync.dma_start(bounceA3, y3[:, :HW, :])
                nc.gpsimd.collective_compute(
                    "AllGather", ALU.bypass, replica_groups=rg,
                    ins=[bounceA[:]], outs=[tabA[:]])

            def ag_b():
                nc.sync.dma_start(bounceB3, y3[:, HW:, :])
                nc.gpsimd.collective_compute(
                    "AllGather", ALU.bypass, replica_groups=rg,
                    ins=[bounceB[:]], outs=[tabB[:]])

            # ---- MLP (bf16) ----
            with (
                tc.tile_pool(name="xload", bufs=3) as xload,
                tc.tile_pool(name="xT", bufs=8) as xTp,
                tc.tile_pool(name="h1", bufs=2) as h1p,
                tc.tile_pool(name="h1T", bufs=4) as h1Tp,
                tc.tile_pool(name="psT", bufs=4, space="PSUM") as psT,
                tc.tile_pool(name="psH", bufs=2, space="PSUM") as psH,
                tc.tile_pool(name="psO", bufs=2, space="PSUM") as psO,
            ):
                ident = constp.tile([128, 128], BF16)
                make_identity(nc, ident[:])
                for t in range(NT):
                    xt = xload.tile([128, cfg.cpad], BF16)
                    nc.sync.dma_start(xt[:], xs_d[t * 128:(t + 1) * 128, :])
                    xTs = []
                    for c in range(NCH):
                        pt = psT.tile([128, 128], BF16)
                        nc.tensor.transpose(pt[:], xt[:, c * 128:(c + 1) * 128],
                                            ident[:])
                        xTc = xTp.tile([128, 128], BF16)
                        if c % 2 == 0:
                            nc.vector.tensor_copy(xTc[:], pt[:])
                        else:
                            nc.scalar.mul(xTc[:], pt[:], 1.0)
                        xTs.append(xTc)
                    h1ps = psH.tile([128, cfg.hid], F32)
                    for c in range(NCH):
                        nc.tensor.matmul(h1ps[:], lhsT=xTs[c][:],
                                         rhs=w1sb[:, c * cfg.hid:(c + 1) * cfg.hid],
                                         start=(c == 0), stop=(c == NCH - 1))
                    h1 = h1p.tile([128, cfg.hid], BF16)
                    nc.scalar.activation(h1[:], h1ps[:], AF.Relu)
                    h1Ts = []
                    for c in range(NHC):
                        pt = psT.tile([128, 128], BF16)
                        nc.tensor.transpose(pt[:], h1[:, c * 128:(c + 1) * 128],
                                            ident[:])
                        hTc = h1Tp.tile([128, 128], BF16)
                        if c % 2 == 0:
                            nc.vector.tensor_copy(hTc[:], pt[:])
                        else:
                            nc.scalar.mul(hTc[:], pt[:], 1.0)
                        h1Ts.append(hTc)
                    hps = psO.tile([128, F], F32)
                    for c in range(NHC):
                        nc.tensor.matmul(hps[:], lhsT=h1Ts[c][:],
                                         rhs=w2sb[:, c * F:(c + 1) * F],
                                         start=(c == 0), stop=(c == NHC - 1))
                    nc.scalar.mul(hid_sb[:, t * F:(t + 1) * F], hps[:], temps[0])
                    nc.vector.tensor_scalar_mul(ysb[:, t * F:(t + 1) * F],
                                                hps[:], dinv_t[:, t:t + 1])
                    if t == HW - 1:
                        ag_a()
                ag_b()

            # ---- propagation steps ----
            with (
                tc.tile_pool(name="msg", bufs=12) as msgp,
                tc.tile_pool(name="oh", bufs=8) as ohp,
                tc.tile_pool(name="tmp", bufs=4) as tmpp,
                tc.tile_pool(name="tw", bufs=4) as twp,
                tc.tile_pool(name="win", bufs=2 * cfg.sw, space="PSUM") as winp,
            ):
                gq = 0
                for s in range(1, n_steps + 1):
                    psums = {}
                    for ci, (h, b0, nb, col0, nco) in enumerate(plan.chunks):
                        L = nb * 128
                        msg = msgp.tile([128, CB, 128], BF16, tag="msg")
                        tbl = tabA_v if h == 0 else tabB_v
                        nc.gpsimd.dma_gather(
                            msg[:, :nb, :], tbl,
                            gidx_sb[:, b0 * 8:b0 * 8 + L // 16], L, L, 128,
                            queue_num=gq % 4)
                        gq += 1
                        oh = ohp.tile([128, MAXC * 128], FP8, tag="oh")
                        oh3 = oh[:].rearrange("p (b f) -> p b f", f=128)
                        nc.vector.tensor_tensor(
                            oh3[:, :nco, :],
                            doff_sb[:, col0:col0 + nco].to_broadcast(
                                [128, nco, 128]),
                            iota3.to_broadcast([128, nco, 128]),
                            ALU.is_equal)
                        for j in range(nb):
                            w, first, last, segs = plan.blocks[b0 + j]
                            if first:
                                psums[w] = winp.tile([128, F], F32, tag="win",
                                                     name=f"w_s{s}_h{h}_{w}")
                            nseg = len(segs)
                            for si, (p, colid) in enumerate(segs):
                                nc.tensor.matmul(
                                    psums[w][:],
                                    lhsT=oh3[:, colid - col0, :],
                                    rhs=msg[:, j, p * 64:(p + 1) * 64],
                                    start=(first and si == 0),
                                    stop=(last and si == nseg - 1))
                            if last:
                                ps = psums.pop(w)
                                if h == 0:
                                    # stash A-pass partial
                                    nc.vector.tensor_copy(st3[:, w, :], ps[:])
                                else:
                                    tmp = tmpp.tile([128, F], F32, tag="tmp")
                                    nc.vector.tensor_add(tmp[:], ps[:],
                                                         st3[:, w, :])
                                    if s < n_steps:
                                        nc.scalar.activation(
                                            y3[:, w, :], tmp[:], AF.Copy,
                                            scale=dinv2_t[:, w:w + 1])
                                    if s % 2 == 0:
                                        m = s // 2
                                        tw = twp.tile([128, F], F32, tag="tw")
                                        nc.scalar.activation(
                                            tw[:], tmp[:], AF.Copy,
                                            scale=tdinv_t[:, (m - 1) * NT + w:
                                                          (m - 1) * NT + w + 1])
                                        nc.vector.tensor_add(
                                            hid3[:, w, :], hid3[:, w, :], tw[:])
                        if ci == plan.ag1_chunk and s < n_steps:
                            ag_a()
                    if s < n_steps:
                        ag_b()

                # ---- log_softmax ----
                with tc.tile_pool(name="soft", bufs=1) as softp:
                    CO = cfg.cout
                    hsl = hid3[:, :, :CO]
                    mx = softp.tile([128, NT], F32, tag="mx")
                    nc.vector.tensor_reduce(mx[:], hsl, mybir.AxisListType.X,
                                            ALU.max)
                    ex = softp.tile([128, NT * F], F32, tag="scratch")
                    ex3 = ex[:].rearrange("p (t f) -> p t f", f=F)[:, :, :CO]
                    nc.vector.tensor_tensor(
                        ex3, hsl, mx[:].to_broadcast([128, NT, CO]),
                        ALU.subtract)
                    nc.scalar.activation(ex3, ex3, AF.Exp)
                    sm = softp.tile([128, NT], F32, tag="sm")
                    nc.vector.tensor_reduce(sm[:], ex3, mybir.AxisListType.X,
                                            ALU.add)
                    ln = softp.tile([128, NT], F32, tag="ln")
                    nc.scalar.activation(ln[:], sm[:], AF.Ln)
                    ml = softp.tile([128, NT], F32, tag="ml")
                    nc.vector.tensor_add(ml[:], mx[:], ln[:])
                    ot = softp.tile([128, NT * CO], F32, tag="ot")
                    ot3 = ot[:].rearrange("p (t f) -> p t f", f=CO)
                    nc.vector.tensor_tensor(
                        ot3, hsl, ml[:].to_broadcast([128, NT, CO]),
                        ALU.subtract)
                    out3 = out_d[:].rearrange("(t p) f -> p t f", p=128)
                    nc.sync.dma_start(out3, ot3)

    nc.compile()
    return nc


# --------------------------------------------------------------------------
# entry point
# --------------------------------------------------------------------------

def kernel_with_results(x, edge_index, W1, b1, W2, b2, temp, trace=False):
    cfg = CFG
    in_maps, plan, temps = preprocess(x, edge_index, W1, b1, W2, b2, temp, cfg)
    nc = build_program(cfg, plan, temps)
    res = run_bass_kernel_spmd(nc, in_maps, core_ids=list(range(cfg.n_cores)),
                               trace=trace)
    outs = [res.results[c]["out"] for c in range(cfg.n_cores)]
    full = np.concatenate(outs, axis=0)[: cfg.n]
    return full.astype(np.float32), res


def kernel(x, edge_index, W1, b1, W2, b2, temp):
    out, _ = kernel_with_results(x, edge_index, W1, b1, W2, b2, temp)
    return out


# revision 5
# speedup vs baseline: 1.1586x; 1.0374x over previous
"""EvenNet (even-order PPR GNN) Trainium2 kernel, 8-core SPMD.

Math: hidden = sum_{k=0..5} temp[k] * A_hat^{2k} @ MLP(x); out = log_softmax(hidden)
with A_hat = D^-1/2 A D^-1/2 (D = out-degree from src indices).

Reformulation: iterate in y-space, y = D^-1/2 x.  One propagation x <- A_hat x
becomes y <- D^-1 * (A y) where (A y)[d] = sum_{e: dst=d} y[src_e] — a pure
unweighted gather + segment-sum.  Per-node scalings fold into window flushes.

Distribution: nodes sharded contiguously across 8 cores; edges routed to the
dst-owner core.  The replicated y-table is split into two DRAM halves, each
interleaving every core's half-shard:
  tableA row c*6272 + l  = y of node c*12544 + l         (l < 6272)
  tableB row c*6272 + l' = y of node c*12544 + 6272 + l'
so pair indices stay < 25088 (int16 dma_gather limit) AND each half can be
AllGathered independently: AG_A fires mid-step (after local windows 0..48
flush), AG_B at step end.  The next step's A-pass gathers depend only on
AG_A, so the Q7 descriptor generator runs straight through the AG_B barrier.

Per step each core runs an A-pass then a B-pass over its 98 dst windows:
gather chunks (1024 edges, 4 SWDGE queues) read one table half; per 128-edge
block a one-hot [edges x 128] fp8 matrix (DVE compare vs iota from
precomputed dst offsets) scatter-sums gathered bf16 pair-rows on the
TensorEngine into the window PSUM.  Blocks mix both parities: edges are
sorted by parity inside each (window, half) group, so most blocks emit one
matmul; the straddling block emits two (one per parity, disjoint one-hot
columns).  A-pass partials stash to SBUF f32; B-pass adds them back and
applies the D^-1 / temp scalings.
"""

import dataclasses
import math
import numpy as np
import ml_dtypes

from concourse import bacc, bass, mybir, tile
from concourse.bass_utils import run_bass_kernel_spmd
from concourse.masks import make_identity

F32 = mybir.dt.float32
BF16 = mybir.dt.bfloat16
FP8 = mybir.dt.float8e4
I16 = mybir.dt.int16
AF = mybir.ActivationFunctionType
ALU = mybir.AluOpType
NPBF16 = ml_dtypes.bfloat16


@dataclasses.dataclass
class Cfg:
    n_cores: int = 8
    n: int = 100000          # real nodes
    cin: int = 500           # input channels
    hid: int = 256           # MLP hidden
    cout: int = 50           # classes
    k_half: int = 5          # outer iterations (2 props each)
    chunk_blocks: int = 8    # 128-edge blocks per dma_gather (1024 = ring cap)
    sw: int = 4              # windows per super-window (PSUM rotation)
    f: int = 64              # padded feature dim

    @property
    def nsh(self):           # nodes per core (padded)
        return int(math.ceil(self.n / self.n_cores / 128) * 128)

    @property
    def nt(self):            # windows per core
        return self.nsh // 128

    @property
    def hw_(self):           # windows in half A
        return (self.nt + 1) // 2

    @property
    def hn(self):            # nodes per half-shard
        return self.hw_ * 128

    @property
    def cpad(self):
        return int(math.ceil(self.cin / 128) * 128)

    @property
    def tab_rows(self):      # rows per table half
        return self.hn * self.n_cores

    @property
    def tab_pairs(self):
        return self.tab_rows // 2


CFG = Cfg()


@dataclasses.dataclass
class Plan:
    blocks: list     # (w, start, stop, segs) per stream block; segs=[(par, col)]
    chunks: list     # (h, b0, nb, col0, ncols)
    nblk: int
    ncols: int
    ag1_chunk: int   # chunk index after which windows 0..hw_-1 have flushed


def _wrap16(arr):
    L = arr.shape[0]
    assert L % 16 == 0
    w = np.ascontiguousarray(arr.reshape(L // 16, 16).T)
    return np.tile(w, (8, 1))


def _tile_major(v, cfg):
    return np.ascontiguousarray(v.reshape(cfg.nt, 128).T)


# --------------------------------------------------------------------------
# host-side preprocessing
# --------------------------------------------------------------------------

def preprocess(x, edge_index, W1, b1, W2, b2, temp, cfg):
    n, NT, HW, HN = cfg.n, cfg.nt, cfg.hw_, cfg.hn
    NSH = cfg.nsh
    src = np.asarray(edge_index[0]).astype(np.int64)
    dst = np.asarray(edge_index[1]).astype(np.int64)
    x = np.asarray(x, dtype=np.float32)
    W1 = np.asarray(W1, dtype=np.float32)
    W2 = np.asarray(W2, dtype=np.float32)
    b1 = np.asarray(b1, dtype=np.float32)
    b2 = np.asarray(b2, dtype=np.float32)
    temp = np.asarray(temp, dtype=np.float32)
    assert not np.any(b1) and not np.any(b2), "bias path removed"

    deg = np.bincount(src, minlength=n).astype(np.float32)
    dinv = np.where(deg > 0, 1.0 / np.sqrt(np.maximum(deg, 1e-12)), 0.0).astype(np.float32)
    dinv2 = (dinv * dinv).astype(np.float32)
    temps = [float(t) for t in temp]

    # ---- per-edge routing ----
    c_s = src // NSH
    l_s = src % NSH
    half = l_s // HN                       # which table
    row = c_s * HN + (l_s % HN)            # row within table half
    pair = row // 2
    par = row % 2

    owner = dst // NSH
    ngrp = NT * 2                          # (w, half)
    counts = np.zeros((cfg.n_cores, ngrp, 2), dtype=np.int64)
    per_core = []
    for c in range(cfg.n_cores):
        m = owner == c
        dl = dst[m] - c * NSH
        w = dl // 128
        gid = w * 2 + half[m]
        o = np.lexsort((pair[m], par[m], gid))
        per_core.append((pair[m][o], (dl % 128)[o], par[m][o], gid[o]))
        cnt2 = np.zeros((ngrp, 2), dtype=np.int64)
        np.add.at(cnt2, (gid[o], par[m][o]), 1)
        counts[c] = cnt2

    tot_cnt = counts.sum(axis=2)           # [cores, ngrp]
    nblocks = np.maximum(np.ceil(tot_cnt.max(axis=0) / 128).astype(np.int64), 1)

    # ---- shared block stream: A-pass then B-pass, sw-rotated windows ----
    blocks = []            # (w, gid) per stream slot
    chunk_bounds = []      # (h, start_block, nb) chunks
    for h in (0, 1):
        pass_start = len(blocks)
        for W0 in range(0, NT, cfg.sw):
            per_w = []
            for w in range(W0, min(W0 + cfg.sw, NT)):
                per_w.append([(w, w * 2 + h)] * int(nblocks[w * 2 + h]))
            while any(per_w):
                for lst in per_w:
                    if lst:
                        blocks.append(lst.pop(0))
        for i in range(pass_start, len(blocks), cfg.chunk_blocks):
            nb = min(cfg.chunk_blocks, len(blocks) - i)
            chunk_bounds.append((h, i, nb))
    nblk = len(blocks)

    # first/last per (w, h) pass segment
    first_seen, last_idx = {}, {}
    for i, (w, gid) in enumerate(blocks):
        if gid not in first_seen:
            first_seen[gid] = i
        last_idx[gid] = i

    # ordered stream positions of each group's blocks (interleaved by the
    # super-window round-robin)
    gblocks = {}
    for i, (w, gid) in enumerate(blocks):
        gblocks.setdefault(gid, []).append(i)

    # ---- segments (parity runs) per block ----
    # Per core, group gid's slots: par0 edges at [0, cnt0), par1 at
    # [cnt0, cnt0+cnt1).  Shared flags OR over cores.
    cnt0 = counts[:, :, 0]                 # per core
    cnt1 = counts[:, :, 1]
    has = np.zeros((nblk, 2), dtype=bool)
    for g in range(ngrp):
        if g not in gblocks:
            continue
        bl = gblocks[g]
        for k, bi in enumerate(bl):
            lo, hi = k * 128, (k + 1) * 128
            for c in range(cfg.n_cores):
                c0 = int(cnt0[c, g])
                ctot = c0 + int(cnt1[c, g])
                if min(c0, hi) > lo:
                    has[bi, 0] = True
                if min(ctot, hi) > max(c0, lo):
                    has[bi, 1] = True
        if not has[bl].any():
            has[bl[0], 0] = True           # empty group: dummy even segment

    # column assignment (one one-hot column per (block, parity) segment)
    segs_per_block = []
    col = 0
    for i in range(nblk):
        segs = [(p, None) for p in (0, 1) if has[i, p]]
        segs = [(p, col + j) for j, (p, _) in enumerate(segs)]
        col += len(segs)
        segs_per_block.append(segs)
    ncols = col

    blocks_meta = []
    for i, (w, gid) in enumerate(blocks):
        blocks_meta.append((w, i == first_seen[gid], i == last_idx[gid],
                            segs_per_block[i]))

    # chunks with column ranges
    chunks = []
    for (h, b0, nb) in chunk_bounds:
        col0 = segs_per_block[b0][0][1]
        last = segs_per_block[b0 + nb - 1]
        colend = last[-1][1] + 1
        chunks.append((h, b0, nb, col0, colend - col0))

    # AG1 point: after the chunk containing the stop-B block of window HW-1
    wtarget = HW - 1
    stopB_blk = last_idx[wtarget * 2 + 1]
    ag1_chunk = next(ci for ci, (h, b0, nb, _, _) in enumerate(chunks)
                     if h == 1 and b0 <= stopB_blk < b0 + nb)

    # ---- per-core index / dstoff arrays ----
    tot = nblk * 128
    gidx_maps, doff_maps = [], []
    for c in range(cfg.n_cores):
        pr, dl, pa, gid = per_core[c]
        g_arr = np.zeros(tot, dtype=np.int16)
        d_arr = np.full((ncols, 128), -1.0, dtype=np.float32)
        gb = np.concatenate([[0], np.cumsum(tot_cnt[c])])
        for g in range(ngrp):
            cnt = int(tot_cnt[c, g])
            if cnt == 0:
                continue
            bl = np.asarray(gblocks[g], dtype=np.int64)
            sl = slice(gb[g], gb[g + 1])
            j = np.arange(cnt)
            blk = bl[j // 128]
            slot = j % 128
            pos = blk * 128 + slot
            g_arr[pos] = pr[sl].astype(np.int16)
            # dstoff goes to the column matching (block, parity)
            par_e = pa[sl]
            for b in np.unique(blk):
                mb = blk == b
                for (p, colid) in segs_per_block[b]:
                    mm = mb & (par_e == p)
                    d_arr[colid, slot[mm]] = dl[sl][mm]
        gidx_maps.append(_wrap16(g_arr))
        doff_maps.append(np.ascontiguousarray(d_arr.T).astype(NPBF16))

    # ---- dense per-core inputs (bf16 MLP) ----
    W1p = np.zeros((cfg.cpad, cfg.hid), dtype=NPBF16)
    W1p[: cfg.cin] = W1.astype(NPBF16)
    W2p = np.zeros((cfg.hid, cfg.f), dtype=NPBF16)
    W2p[:, : cfg.cout] = W2.astype(NPBF16)
    iota = np.tile(np.arange(128, dtype=np.float32)[None, :], (128, 1)).astype(NPBF16)
    in_maps = []
    for c in range(cfg.n_cores):
        lo = c * NSH
        real = max(0, min(NSH, n - lo))
        xs = np.zeros((NSH, cfg.cpad), dtype=NPBF16)
        if real > 0:
            xs[:real, : cfg.cin] = x[lo:lo + real].astype(NPBF16)
        dv = np.zeros(NSH, np.float32)
        dv[:real] = dinv[lo:lo + real]
        dv2 = np.zeros(NSH, np.float32)
        dv2[:real] = dinv2[lo:lo + real]
        tdv = np.zeros((128, cfg.k_half * NT), np.float32)
        for m in range(1, cfg.k_half + 1):
            tdv[:, (m - 1) * NT:m * NT] = _tile_major(
                (temps[m] * dv).astype(np.float32), cfg)
        in_maps.append({
            "xs": xs,
            "w1": W1p,
            "w2": W2p,
            "gidx": gidx_maps[c],
            "dstoff": doff_maps[c],
            "iota": iota,
            "dinv_t": _tile_major(dv, cfg),
            "dinv2_t": _tile_major(dv2, cfg),
            "tdinv": tdv,
        })

    plan = Plan(blocks=blocks_meta, chunks=chunks, nblk=nblk, ncols=ncols,
                ag1_chunk=ag1_chunk)
    return in_maps, plan, temps


# --------------------------------------------------------------------------
# program builder
# --------------------------------------------------------------------------

def build_program(cfg, plan, temps):
    nc = bacc.Bacc("TRN2", target_bir_lowering=False, debug=False,
                   num_devices=cfg.n_cores, num_swdge_queues=4)

    NT, F, NSH, HW, HN = cfg.nt, cfg.f, cfg.nsh, cfg.hw_, cfg.hn
    NCH, NHC = cfg.cpad // 128, cfg.hid // 128
    nblk, ncols = plan.nblk, plan.ncols
    CB = cfg.chunk_blocks
    MAXC = 2 * CB

    xs_d = nc.declare_dram_parameter("xs", [NSH, cfg.cpad], BF16, isOutput=False)
    w1_d = nc.declare_dram_parameter("w1", [cfg.cpad, cfg.hid], BF16, isOutput=False)
    w2_d = nc.declare_dram_parameter("w2", [cfg.hid, F], BF16, isOutput=False)
    gidx_d = nc.declare_dram_parameter("gidx", [128, nblk * 8], I16, isOutput=False)
    doff_d = nc.declare_dram_parameter("dstoff", [128, ncols], BF16, isOutput=False)
    iota_d = nc.declare_dram_parameter("iota", [128, 128], BF16, isOutput=False)
    dinv_d = nc.declare_dram_parameter("dinv_t", [128, NT], F32, isOutput=False)
    dinv2_d = nc.declare_dram_parameter("dinv2_t", [128, NT], F32, isOutput=False)
    tdinv_d = nc.declare_dram_parameter("tdinv", [128, cfg.k_half * NT], F32,
                                        isOutput=False)
    out_d = nc.declare_dram_parameter("out", [NSH, cfg.cout], F32, isOutput=True)

    tabA = nc.dram_tensor("ytabA", [cfg.tab_rows, F], BF16, addr_space="Shared")
    tabB = nc.dram_tensor("ytabB", [cfg.tab_rows, F], BF16, addr_space="Shared")
    bounceA = nc.dram_tensor("ybounceA", [HN, F], BF16)
    bounceB = nc.dram_tensor("ybounceB", [HN, F], BF16)
    tabA_v = tabA[:].rearrange("(a b) f -> a (b f)", b=2)   # [pairs, 128]
    tabB_v = tabB[:].rearrange("(a b) f -> a (b f)", b=2)

    n_steps = 2 * cfg.k_half
    rg = [list(range(cfg.n_cores))]

    with tile.TileContext(nc) as tc:
        with (
            tc.tile_pool(name="const", bufs=1) as constp,
            tc.tile_pool(name="persist", bufs=1) as persist,
        ):
            w1sb = constp.tile([128, NCH * cfg.hid], BF16)
            for c in range(NCH):
                nc.sync.dma_start(w1sb[:, c * cfg.hid:(c + 1) * cfg.hid],
                                  w1_d[c * 128:(c + 1) * 128, :])
            w2sb = constp.tile([128, NHC * F], BF16)
            for c in range(NHC):
                nc.sync.dma_start(w2sb[:, c * F:(c + 1) * F],
                                  w2_d[c * 128:(c + 1) * 128, :])
            dinv_t = constp.tile([128, NT], F32)
            nc.sync.dma_start(dinv_t[:], dinv_d[:])
            dinv2_t = constp.tile([128, NT], F32)
            nc.sync.dma_start(dinv2_t[:], dinv2_d[:])
            tdinv_t = constp.tile([128, cfg.k_half * NT], F32)
            nc.sync.dma_start(tdinv_t[:], tdinv_d[:])
            doff_sb = constp.tile([128, ncols], BF16)
            nc.sync.dma_start(doff_sb[:], doff_d[:])
            gidx_sb = constp.tile([128, nblk * 8], I16)
            nc.sync.dma_start(gidx_sb[:], gidx_d[:])
            iota_sb = constp.tile([128, 128], BF16)
            nc.sync.dma_start(iota_sb[:], iota_d[:])
            iota3 = iota_sb[:].rearrange("p (a f) -> p a f", a=1)

            hid_sb = persist.tile([128, NT * F], F32)
            ysb = persist.tile([128, NT * F], BF16)
            stash = persist.tile([128, NT * F], F32)
            hid3 = hid_sb[:].rearrange("p (t f) -> p t f", f=F)
            y3 = ysb[:].rearrange("p (t f) -> p t f", f=F)
            st3 = stash[:].rearrange("p (t f) -> p t f", f=F)

            bounceA3 = bounceA[:].rearrange("(t p) f -> p t f", p=128)
            bounceB3 = bounceB[:].rearrange("(t p) f -> p t f", p=128)

            def ag_a():
                nc.sync.dma_start(bounceA3, y3[:, :HW, :])
                nc.gpsimd.collective_compute(
                    "AllGather", ALU.bypass, replica_groups=rg,
                    ins=[bounceA[:]], outs=[tabA[:]])

            def ag_b():
                nc.sync.dma_start(bounceB3, y3[:, HW:, :])
                nc.gpsimd.collective_compute(
                    "AllGather", ALU.bypass, replica_groups=rg,
                    ins=[bounceB[:]], outs=[tabB[:]])

            # ---- MLP (bf16) ----
            with (
                tc.tile_pool(name="xload", bufs=3) as xload,
                tc.tile_pool(name="xT", bufs=8) as xTp,
                tc.tile_pool(name="h1", bufs=2) as h1p,
                tc.tile_pool(name="h1T", bufs=4) as h1Tp,
                tc.tile_pool(name="psT", bufs=4, space="PSUM") as psT,
                tc.tile_pool(name="psH", bufs=2, space="PSUM") as psH,
                tc.tile_pool(name="psO", bufs=2, space="PSUM") as psO,
            ):
                ident = constp.tile([128, 128], BF16)
                make_identity(nc, ident[:])
                for t in range(NT):
                    xt = xload.tile([128, cfg.cpad], BF16)
                    nc.sync.dma_start(xt[:], xs_d[t * 128:(t + 1) * 128, :])
                    xTs = []
                    for c in range(NCH):
                        pt = psT.tile([128, 128], BF16)
                        nc.tensor.transpose(pt[:], xt[:, c * 128:(c + 1) * 128],
                                            ident[:])
                        xTc = xTp.tile([128, 128], BF16)
                        if c % 2 == 0:
                            nc.vector.tensor_copy(xTc[:], pt[:])
                        else:
                            nc.scalar.mul(xTc[:], pt[:], 1.0)
                        xTs.append(xTc)
                    h1ps = psH.tile([128, cfg.hid], F32)
                    for c in range(NCH):
                        nc.tensor.matmul(h1ps[:], lhsT=xTs[c][:],
                                         rhs=w1sb[:, c * cfg.hid:(c + 1) * cfg.hid],
                                         start=(c == 0), stop=(c == NCH - 1))
                    h1 = h1p.tile([128, cfg.hid], BF16)
                    nc.scalar.activation(h1[:], h1ps[:], AF.Relu)
                    h1Ts = []
                    for c in range(NHC):
                        pt = psT.tile([128, 128], BF16)
                        nc.tensor.transpose(pt[:], h1[:, c * 128:(c + 1) * 128],
                                            ident[:])
                        hTc = h1Tp.tile([128, 128], BF16)
                        if c % 2 == 0:
                            nc.vector.tensor_copy(hTc[:], pt[:])
                        else:
                            nc.scalar.mul(hTc[:], pt[:], 1.0)
                        h1Ts.append(hTc)
                    hps = psO.tile([128, F], F32)
                    for c in range(NHC):
                        nc.tensor.matmul(hps[:], lhsT=h1Ts[c][:],
                                         rhs=w2sb[:, c * F:(c + 1) * F],
                                         start=(c == 0), stop=(c == NHC - 1))
                    nc.scalar.mul(hid_sb[:, t * F:(t + 1) * F], hps[:], temps[0])
                    nc.vector.tensor_scalar_mul(ysb[:, t * F:(t + 1) * F],
                                                hps[:], dinv_t[:, t:t + 1])
                    if t == HW - 1:
                        ag_a()
                ag_b()

            # ---- propagation steps ----
            with (
                tc.tile_pool(name="msg", bufs=12) as msgp,
                tc.tile_pool(name="oh", bufs=8) as ohp,
                tc.tile_pool(name="tmp", bufs=4) as tmpp,
                tc.tile_pool(name="tw", bufs=4) as twp,
                tc.tile_pool(name="win", bufs=2 * cfg.sw, space="PSUM") as winp,
            ):
                gq = 0
                for s in range(1, n_steps + 1):
                    psums = {}
                    for ci, (h, b0, nb, col0, nco) in enumerate(plan.chunks):
                        L = nb * 128
                        msg = msgp.tile([128, CB, 128], BF16, tag="msg")
                        tbl = tabA_v if h == 0 else tabB_v
                        nc.gpsimd.dma_gather(
                            msg[:, :nb, :], tbl,
                            gidx_sb[:, b0 * 8:b0 * 8 + L // 16], L, L, 128,
                            queue_num=gq % 4)
                        gq += 1
                        oh = ohp.tile([128, MAXC * 128], FP8, tag="oh")
                        oh3 = oh[:].rearrange("p (b f) -> p b f", f=128)
                        nc.vector.tensor_tensor(
                            oh3[:, :1, :],
                            doff_sb[:, col0:col0 + 1].to_broadcast(
                                [128, 1, 128]),
                            iota3.to_broadcast([128, 1, 128]),
                            ALU.is_equal)
                        for j in range(nb):
                            w, first, last, segs = plan.blocks[b0 + j]
                            if first:
                                psums[w] = winp.tile([128, F], F32, tag="win",
                                                     name=f"w_s{s}_h{h}_{w}")
                            nseg = len(segs)
                            for si, (p, colid) in enumerate(segs):
                                nc.tensor.matmul(
                                    psums[w][:],
                                    lhsT=oh3[:, 0, :],
                                    rhs=msg[:, j, p * 64:(p + 1) * 64],
                                    start=(first and si == 0),
                                    stop=(last and si == nseg - 1))
                            if last:
                                ps = psums.pop(w)
                                if h == 0:
                                    # stash A-pass partial
                                    nc.vector.tensor_copy(st3[:, w, :], ps[:])
                                else:
                                    tmp = tmpp.tile([128, F], F32, tag="tmp")
                                    nc.vector.tensor_add(tmp[:], ps[:],
                                                         st3[:, w, :])
                                    if s < n_steps:
                                        nc.scalar.activation(
                                            y3[:, w, :], tmp[:], AF.Copy,
                                            scale=dinv2_t[:, w:w + 1])
                                    if s % 2 == 0:
                                        m = s // 2
                                        tw = twp.tile([128, F], F32, tag="tw")
                                        nc.scalar.activation(
                                            tw[:], tmp[:], AF.Copy,
                                            scale=tdinv_t[:, (m - 1) * NT + w:
                                                          (m - 1) * NT + w + 1])
                                        nc.vector.tensor_add(
                                            hid3[:, w, :], hid3[:, w, :], tw[:])
                        if ci == plan.ag1_chunk and s < n_steps:
                            ag_a()
                    if s < n_steps:
                        ag_b()

                # ---- log_softmax ----
                with tc.tile_pool(name="soft", bufs=1) as softp:
                    CO = cfg.cout
                    hsl = hid3[:, :, :CO]
                    mx = softp.tile([128, NT], F32, tag="mx")
                    nc.vector.tensor_reduce(mx[:], hsl, mybir.AxisListType.X,
                                            ALU.max)
                    ex = softp.tile([128, NT * F], F32, tag="scratch")
                    ex3 = ex[:].rearrange("p (t f) -> p t f", f=F)[:, :, :CO]
                    nc.vector.tensor_tensor(
                        ex3, hsl, mx[:].to_broadcast([128, NT, CO]),
                        ALU.subtract)
                    nc.scalar.activation(ex3, ex3, AF.Exp)
                    sm = softp.tile([128, NT], F32, tag="sm")
                    nc.vector.tensor_reduce(sm[:], ex3, mybir.AxisListType.X,
                                            ALU.add)
                    ln = softp.tile([128, NT], F32, tag="ln")
                    nc.scalar.activation(ln[:], sm[:], AF.Ln)
                    ml = softp.tile([128, NT], F32, tag="ml")
                    nc.vector.tensor_add(ml[:], mx[:], ln[:])
                    ot = softp.tile([128, NT * CO], F32, tag="ot")
                    ot3 = ot[:].rearrange("p (t f) -> p t f", f=CO)
                    nc.vector.tensor_tensor(
                        ot3, hsl, ml[:].to_broadcast([128, NT, CO]),
                        ALU.subtract)
                    out3 = out_d[:].rearrange("(t p) f -> p t f", p=128)
                    nc.sync.dma_start(out3, ot3)

    nc.compile()
    return nc


# --------------------------------------------------------------------------
# entry point
# --------------------------------------------------------------------------

def kernel_with_results(x, edge_index, W1, b1, W2, b2, temp, trace=False):
    cfg = CFG
    in_maps, plan, temps = preprocess(x, edge_index, W1, b1, W2, b2, temp, cfg)
    nc = build_program(cfg, plan, temps)
    res = run_bass_kernel_spmd(nc, in_maps, core_ids=list(range(cfg.n_cores)),
                               trace=trace)
    outs = [res.results[c]["out"] for c in range(cfg.n_cores)]
    full = np.concatenate(outs, axis=0)[: cfg.n]
    return full.astype(np.float32), res


def kernel(x, edge_index, W1, b1, W2, b2, temp):
    out, _ = kernel_with_results(x, edge_index, W1, b1, W2, b2, temp)
    return out


# revision 6
# speedup vs baseline: 1.2186x; 1.0518x over previous
"""EvenNet (even-order PPR GNN) Trainium2 kernel, 8-core SPMD.

Math: hidden = sum_{k=0..5} temp[k] * A_hat^{2k} @ MLP(x); out = log_softmax(hidden)
with A_hat = D^-1/2 A D^-1/2 (D = out-degree from src indices).

Reformulation: iterate in y-space, y = D^-1/2 x.  One propagation x <- A_hat x
becomes y <- D^-1 * (A y) where (A y)[d] = sum_{e: dst=d} y[src_e] — a pure
unweighted gather + segment-sum.  Per-node scalings fold into window flushes.

Distribution: nodes sharded contiguously across 8 cores; edges routed to the
dst-owner core.  The replicated y-table is split into two DRAM halves, each
interleaving every core's half-shard:
  tableA row c*6272 + l  = y of node c*12544 + l         (l < 6272)
  tableB row c*6272 + l' = y of node c*12544 + 6272 + l'
so pair indices stay < 25088 (int16 dma_gather limit) AND each half can be
AllGathered independently: AG_A fires mid-step (after local windows 0..48
flush), AG_B at step end.  The next step's A-pass gathers depend only on
AG_A, so the Q7 descriptor generator runs straight through the AG_B barrier.

Per step each core runs an A-pass then a B-pass over its 98 dst windows:
gather chunks (1024 edges, 4 SWDGE queues) read one table half; per 128-edge
block a one-hot [edges x 128] fp8 matrix (DVE compare vs iota from
precomputed dst offsets) scatter-sums gathered bf16 pair-rows on the
TensorEngine into the window PSUM.  Blocks mix both parities: edges are
sorted by parity inside each (window, half) group, so most blocks emit one
matmul; the straddling block emits two (one per parity, disjoint one-hot
columns).  A-pass partials stash to SBUF f32; B-pass adds them back and
applies the D^-1 / temp scalings.
"""

import dataclasses
import math
import numpy as np
import ml_dtypes

from concourse import bacc, bass, mybir, tile
from concourse.bass_utils import run_bass_kernel_spmd
from concourse.masks import make_identity

F32 = mybir.dt.float32
BF16 = mybir.dt.bfloat16
FP8 = mybir.dt.float8e4
I16 = mybir.dt.int16
AF = mybir.ActivationFunctionType
ALU = mybir.AluOpType
NPBF16 = ml_dtypes.bfloat16


@dataclasses.dataclass
class Cfg:
    n_cores: int = 8
    n: int = 100000          # real nodes
    cin: int = 500           # input channels
    hid: int = 256           # MLP hidden
    cout: int = 50           # classes
    k_half: int = 5          # outer iterations (2 props each)
    chunk_blocks: int = 8    # 128-edge blocks per dma_gather (1024 = ring cap)
    sw: int = 4              # windows per super-window (PSUM rotation)
    f: int = 64              # padded feature dim

    @property
    def nsh(self):           # nodes per core (padded)
        return int(math.ceil(self.n / self.n_cores / 128) * 128)

    @property
    def nt(self):            # windows per core
        return self.nsh // 128

    @property
    def hw_(self):           # windows in half A
        return (self.nt + 1) // 2

    @property
    def hn(self):            # nodes per half-shard
        return self.hw_ * 128

    @property
    def cpad(self):
        return int(math.ceil(self.cin / 128) * 128)

    @property
    def tab_rows(self):      # rows per table half
        return self.hn * self.n_cores

    @property
    def tab_pairs(self):
        return self.tab_rows // 2


CFG = Cfg()


@dataclasses.dataclass
class Plan:
    blocks: list     # (w, start, stop, segs) per stream block; segs=[(par, col)]
    chunks: list     # (h, b0, nb, col0, ncols)
    nblk: int
    ncols: int
    ag1_chunk: int   # chunk index after which windows 0..hw_-1 have flushed


def _wrap16(arr):
    L = arr.shape[0]
    assert L % 16 == 0
    w = np.ascontiguousarray(arr.reshape(L // 16, 16).T)
    return np.tile(w, (8, 1))


def _tile_major(v, cfg):
    return np.ascontiguousarray(v.reshape(cfg.nt, 128).T)


# --------------------------------------------------------------------------
# host-side preprocessing
# --------------------------------------------------------------------------

def preprocess(x, edge_index, W1, b1, W2, b2, temp, cfg):
    n, NT, HW, HN = cfg.n, cfg.nt, cfg.hw_, cfg.hn
    NSH = cfg.nsh
    src = np.asarray(edge_index[0]).astype(np.int64)
    dst = np.asarray(edge_index[1]).astype(np.int64)
    x = np.asarray(x, dtype=np.float32)
    W1 = np.asarray(W1, dtype=np.float32)
    W2 = np.asarray(W2, dtype=np.float32)
    b1 = np.asarray(b1, dtype=np.float32)
    b2 = np.asarray(b2, dtype=np.float32)
    temp = np.asarray(temp, dtype=np.float32)
    assert not np.any(b1) and not np.any(b2), "bias path removed"

    deg = np.bincount(src, minlength=n).astype(np.float32)
    dinv = np.where(deg > 0, 1.0 / np.sqrt(np.maximum(deg, 1e-12)), 0.0).astype(np.float32)
    dinv2 = (dinv * dinv).astype(np.float32)
    temps = [float(t) for t in temp]

    # ---- per-edge routing ----
    c_s = src // NSH
    l_s = src % NSH
    half = l_s // HN                       # which table
    row = c_s * HN + (l_s % HN)            # row within table half
    pair = row // 2
    par = row % 2

    owner = dst // NSH
    ngrp = NT * 2                          # (w, half)
    counts = np.zeros((cfg.n_cores, ngrp, 2), dtype=np.int64)
    per_core = []
    for c in range(cfg.n_cores):
        m = owner == c
        dl = dst[m] - c * NSH
        w = dl // 128
        gid = w * 2 + half[m]
        o = np.lexsort((pair[m], par[m], gid))
        per_core.append((pair[m][o], (dl % 128)[o], par[m][o], gid[o]))
        cnt2 = np.zeros((ngrp, 2), dtype=np.int64)
        np.add.at(cnt2, (gid[o], par[m][o]), 1)
        counts[c] = cnt2

    tot_cnt = counts.sum(axis=2)           # [cores, ngrp]
    nblocks = np.maximum(np.ceil(tot_cnt.max(axis=0) / 128).astype(np.int64), 1)

    # ---- shared block stream: A-pass then B-pass, sw-rotated windows ----
    blocks = []            # (w, gid) per stream slot
    chunk_bounds = []      # (h, start_block, nb) chunks
    for h in (0, 1):
        pass_start = len(blocks)
        for W0 in range(0, NT, cfg.sw):
            per_w = []
            for w in range(W0, min(W0 + cfg.sw, NT)):
                per_w.append([(w, w * 2 + h)] * int(nblocks[w * 2 + h]))
            while any(per_w):
                for lst in per_w:
                    if lst:
                        blocks.append(lst.pop(0))
        for i in range(pass_start, len(blocks), cfg.chunk_blocks):
            nb = min(cfg.chunk_blocks, len(blocks) - i)
            chunk_bounds.append((h, i, nb))
    nblk = len(blocks)

    # first/last per (w, h) pass segment
    first_seen, last_idx = {}, {}
    for i, (w, gid) in enumerate(blocks):
        if gid not in first_seen:
            first_seen[gid] = i
        last_idx[gid] = i

    # ordered stream positions of each group's blocks (interleaved by the
    # super-window round-robin)
    gblocks = {}
    for i, (w, gid) in enumerate(blocks):
        gblocks.setdefault(gid, []).append(i)

    # ---- segments (parity runs) per block ----
    # Per core, group gid's slots: par0 edges at [0, cnt0), par1 at
    # [cnt0, cnt0+cnt1).  Shared flags OR over cores.
    cnt0 = counts[:, :, 0]                 # per core
    cnt1 = counts[:, :, 1]
    has = np.zeros((nblk, 2), dtype=bool)
    for g in range(ngrp):
        if g not in gblocks:
            continue
        bl = gblocks[g]
        for k, bi in enumerate(bl):
            lo, hi = k * 128, (k + 1) * 128
            for c in range(cfg.n_cores):
                c0 = int(cnt0[c, g])
                ctot = c0 + int(cnt1[c, g])
                if min(c0, hi) > lo:
                    has[bi, 0] = True
                if min(ctot, hi) > max(c0, lo):
                    has[bi, 1] = True
        if not has[bl].any():
            has[bl[0], 0] = True           # empty group: dummy even segment

    # column assignment (one one-hot column per (block, parity) segment)
    segs_per_block = []
    col = 0
    for i in range(nblk):
        segs = [(p, None) for p in (0, 1) if has[i, p]]
        segs = [(p, col + j) for j, (p, _) in enumerate(segs)]
        col += len(segs)
        segs_per_block.append(segs)
    ncols = col

    blocks_meta = []
    for i, (w, gid) in enumerate(blocks):
        blocks_meta.append((w, i == first_seen[gid], i == last_idx[gid],
                            segs_per_block[i]))

    # chunks with column ranges
    chunks = []
    for (h, b0, nb) in chunk_bounds:
        col0 = segs_per_block[b0][0][1]
        last = segs_per_block[b0 + nb - 1]
        colend = last[-1][1] + 1
        chunks.append((h, b0, nb, col0, colend - col0))

    # AG1 point: after the chunk containing the stop-B block of window HW-1
    wtarget = HW - 1
    stopB_blk = last_idx[wtarget * 2 + 1]
    ag1_chunk = next(ci for ci, (h, b0, nb, _, _) in enumerate(chunks)
                     if h == 1 and b0 <= stopB_blk < b0 + nb)

    # ---- per-core index / dstoff arrays ----
    tot = nblk * 128
    gidx_maps, doff_maps = [], []
    for c in range(cfg.n_cores):
        pr, dl, pa, gid = per_core[c]
        g_arr = np.zeros(tot, dtype=np.int16)
        d_arr = np.full((ncols, 128), -1.0, dtype=np.float32)
        gb = np.concatenate([[0], np.cumsum(tot_cnt[c])])
        for g in range(ngrp):
            cnt = int(tot_cnt[c, g])
            if cnt == 0:
                continue
            bl = np.asarray(gblocks[g], dtype=np.int64)
            sl = slice(gb[g], gb[g + 1])
            j = np.arange(cnt)
            blk = bl[j // 128]
            slot = j % 128
            pos = blk * 128 + slot
            g_arr[pos] = pr[sl].astype(np.int16)
            # dstoff goes to the column matching (block, parity)
            par_e = pa[sl]
            for b in np.unique(blk):
                mb = blk == b
                for (p, colid) in segs_per_block[b]:
                    mm = mb & (par_e == p)
                    d_arr[colid, slot[mm]] = dl[sl][mm]
        gidx_maps.append(_wrap16(g_arr))
        doff_maps.append(np.ascontiguousarray(d_arr.T).astype(NPBF16))

    # ---- dense per-core inputs (bf16 MLP) ----
    W1p = np.zeros((cfg.cpad, cfg.hid), dtype=NPBF16)
    W1p[: cfg.cin] = W1.astype(NPBF16)
    W2p = np.zeros((cfg.hid, cfg.f), dtype=NPBF16)
    W2p[:, : cfg.cout] = W2.astype(NPBF16)
    iota = np.tile(np.arange(128, dtype=np.float32)[None, :], (128, 1)).astype(NPBF16)
    in_maps = []
    for c in range(cfg.n_cores):
        lo = c * NSH
        real = max(0, min(NSH, n - lo))
        xs = np.zeros((NSH, cfg.cpad), dtype=NPBF16)
        if real > 0:
            xs[:real, : cfg.cin] = x[lo:lo + real].astype(NPBF16)
        dv = np.zeros(NSH, np.float32)
        dv[:real] = dinv[lo:lo + real]
        dv2 = np.zeros(NSH, np.float32)
        dv2[:real] = dinv2[lo:lo + real]
        tdv = np.zeros((128, cfg.k_half * NT), np.float32)
        for m in range(1, cfg.k_half + 1):
            tdv[:, (m - 1) * NT:m * NT] = _tile_major(
                (temps[m] * dv).astype(np.float32), cfg)
        in_maps.append({
            "xs": xs,
            "w1": W1p,
            "w2": W2p,
            "gidx": gidx_maps[c],
            "dstoff": doff_maps[c],
            "iota": iota,
            "dinv_t": _tile_major(dv, cfg),
            "dinv2_t": _tile_major(dv2, cfg),
            "tdinv": tdv,
        })

    plan = Plan(blocks=blocks_meta, chunks=chunks, nblk=nblk, ncols=ncols,
                ag1_chunk=ag1_chunk)
    return in_maps, plan, temps


# --------------------------------------------------------------------------
# program builder
# --------------------------------------------------------------------------

def build_program(cfg, plan, temps):
    nc = bacc.Bacc("TRN2", target_bir_lowering=False, debug=False,
                   num_devices=cfg.n_cores, num_swdge_queues=4)

    NT, F, NSH, HW, HN = cfg.nt, cfg.f, cfg.nsh, cfg.hw_, cfg.hn
    NCH, NHC = cfg.cpad // 128, cfg.hid // 128
    nblk, ncols = plan.nblk, plan.ncols
    CB = cfg.chunk_blocks
    MAXC = 2 * CB

    xs_d = nc.declare_dram_parameter("xs", [NSH, cfg.cpad], BF16, isOutput=False)
    w1_d = nc.declare_dram_parameter("w1", [cfg.cpad, cfg.hid], BF16, isOutput=False)
    w2_d = nc.declare_dram_parameter("w2", [cfg.hid, F], BF16, isOutput=False)
    gidx_d = nc.declare_dram_parameter("gidx", [128, nblk * 8], I16, isOutput=False)
    doff_d = nc.declare_dram_parameter("dstoff", [128, ncols], BF16, isOutput=False)
    iota_d = nc.declare_dram_parameter("iota", [128, 128], BF16, isOutput=False)
    dinv_d = nc.declare_dram_parameter("dinv_t", [128, NT], F32, isOutput=False)
    dinv2_d = nc.declare_dram_parameter("dinv2_t", [128, NT], F32, isOutput=False)
    tdinv_d = nc.declare_dram_parameter("tdinv", [128, cfg.k_half * NT], F32,
                                        isOutput=False)
    out_d = nc.declare_dram_parameter("out", [NSH, cfg.cout], F32, isOutput=True)

    tabA = nc.dram_tensor("ytabA", [cfg.tab_rows, F], BF16, addr_space="Shared")
    tabB = nc.dram_tensor("ytabB", [cfg.tab_rows, F], BF16, addr_space="Shared")
    bounceA = nc.dram_tensor("ybounceA", [HN, F], BF16)
    bounceB = nc.dram_tensor("ybounceB", [HN, F], BF16)
    tabA_v = tabA[:].rearrange("(a b) f -> a (b f)", b=2)   # [pairs, 128]
    tabB_v = tabB[:].rearrange("(a b) f -> a (b f)", b=2)

    n_steps = 2 * cfg.k_half
    rg = [list(range(cfg.n_cores))]

    with tile.TileContext(nc) as tc:
        with (
            tc.tile_pool(name="const", bufs=1) as constp,
            tc.tile_pool(name="persist", bufs=1) as persist,
        ):
            w1sb = constp.tile([128, NCH * cfg.hid], BF16)
            for c in range(NCH):
                nc.sync.dma_start(w1sb[:, c * cfg.hid:(c + 1) * cfg.hid],
                                  w1_d[c * 128:(c + 1) * 128, :])
            w2sb = constp.tile([128, NHC * F], BF16)
            for c in range(NHC):
                nc.sync.dma_start(w2sb[:, c * F:(c + 1) * F],
                                  w2_d[c * 128:(c + 1) * 128, :])
            dinv_t = constp.tile([128, NT], F32)
            nc.sync.dma_start(dinv_t[:], dinv_d[:])
            dinv2_t = constp.tile([128, NT], F32)
            nc.sync.dma_start(dinv2_t[:], dinv2_d[:])
            tdinv_t = constp.tile([128, cfg.k_half * NT], F32)
            nc.sync.dma_start(tdinv_t[:], tdinv_d[:])
            doff_sb = constp.tile([128, ncols], BF16)
            nc.sync.dma_start(doff_sb[:], doff_d[:])
            gidx_sb = constp.tile([128, nblk * 8], I16)
            nc.sync.dma_start(gidx_sb[:], gidx_d[:])
            iota_sb = constp.tile([128, 128], BF16)
            nc.sync.dma_start(iota_sb[:], iota_d[:])
            iota3 = iota_sb[:].rearrange("p (a f) -> p a f", a=1)

            hid_sb = persist.tile([128, NT * F], F32)
            ysb = persist.tile([128, NT * F], BF16)
            stash = persist.tile([128, NT * F], F32)
            hid3 = hid_sb[:].rearrange("p (t f) -> p t f", f=F)
            y3 = ysb[:].rearrange("p (t f) -> p t f", f=F)
            st3 = stash[:].rearrange("p (t f) -> p t f", f=F)

            bounceA3 = bounceA[:].rearrange("(t p) f -> p t f", p=128)
            bounceB3 = bounceB[:].rearrange("(t p) f -> p t f", p=128)

            def ag_a():
                nc.sync.dma_start(bounceA3, y3[:, :HW, :])
                nc.gpsimd.collective_compute(
                    "AllGather", ALU.bypass, replica_groups=rg,
                    ins=[bounceA[:]], outs=[tabA[:]])

            def ag_b():
                nc.sync.dma_start(bounceB3, y3[:, HW:, :])
                nc.gpsimd.collective_compute(
                    "AllGather", ALU.bypass, replica_groups=rg,
                    ins=[bounceB[:]], outs=[tabB[:]])

            # ---- MLP (bf16) ----
            with (
                tc.tile_pool(name="xload", bufs=3) as xload,
                tc.tile_pool(name="xT", bufs=8) as xTp,
                tc.tile_pool(name="h1", bufs=2) as h1p,
                tc.tile_pool(name="h1T", bufs=4) as h1Tp,
                tc.tile_pool(name="psT", bufs=4, space="PSUM") as psT,
                tc.tile_pool(name="psH", bufs=2, space="PSUM") as psH,
                tc.tile_pool(name="psO", bufs=2, space="PSUM") as psO,
            ):
                ident = constp.tile([128, 128], BF16)
                make_identity(nc, ident[:])
                for t in range(NT):
                    xt = xload.tile([128, cfg.cpad], BF16)
                    nc.sync.dma_start(xt[:], xs_d[t * 128:(t + 1) * 128, :])
                    xTs = []
                    for c in range(NCH):
                        pt = psT.tile([128, 128], BF16)
                        nc.tensor.transpose(pt[:], xt[:, c * 128:(c + 1) * 128],
                                            ident[:])
                        xTc = xTp.tile([128, 128], BF16)
                        if c % 2 == 0:
                            nc.vector.tensor_copy(xTc[:], pt[:])
                        else:
                            nc.scalar.mul(xTc[:], pt[:], 1.0)
                        xTs.append(xTc)
                    h1ps = psH.tile([128, cfg.hid], F32)
                    for c in range(NCH):
                        nc.tensor.matmul(h1ps[:], lhsT=xTs[c][:],
                                         rhs=w1sb[:, c * cfg.hid:(c + 1) * cfg.hid],
                                         start=(c == 0), stop=(c == NCH - 1))
                    h1 = h1p.tile([128, cfg.hid], BF16)
                    nc.scalar.activation(h1[:], h1ps[:], AF.Relu)
                    h1Ts = []
                    for c in range(NHC):
                        pt = psT.tile([128, 128], BF16)
                        nc.tensor.transpose(pt[:], h1[:, c * 128:(c + 1) * 128],
                                            ident[:])
                        hTc = h1Tp.tile([128, 128], BF16)
                        if c % 2 == 0:
                            nc.vector.tensor_copy(hTc[:], pt[:])
                        else:
                            nc.scalar.mul(hTc[:], pt[:], 1.0)
                        h1Ts.append(hTc)
                    hps = psO.tile([128, F], F32)
                    for c in range(NHC):
                        nc.tensor.matmul(hps[:], lhsT=h1Ts[c][:],
                                         rhs=w2sb[:, c * F:(c + 1) * F],
                                         start=(c == 0), stop=(c == NHC - 1))
                    nc.scalar.mul(hid_sb[:, t * F:(t + 1) * F], hps[:], temps[0])
                    nc.vector.tensor_scalar_mul(ysb[:, t * F:(t + 1) * F],
                                                hps[:], dinv_t[:, t:t + 1])
                    if t == HW - 1:
                        ag_a()
                ag_b()

            # ---- propagation steps ----
            with (
                tc.tile_pool(name="msg", bufs=12) as msgp,
                tc.tile_pool(name="oh", bufs=8) as ohp,
                tc.tile_pool(name="tmp", bufs=4) as tmpp,
                tc.tile_pool(name="tw", bufs=4) as twp,
                tc.tile_pool(name="win", bufs=2 * cfg.sw, space="PSUM") as winp,
            ):
                gq = 0
                for s in range(1, n_steps + 1):
                    psums = {}
                    for ci, (h, b0, nb, col0, nco) in enumerate(plan.chunks):
                        L = nb * 128
                        msg = msgp.tile([128, CB, 128], BF16, tag="msg")
                        tbl = tabA_v if h == 0 else tabB_v
                        nc.gpsimd.dma_gather(
                            msg[:, :nb, :], tbl,
                            gidx_sb[:, b0 * 8:b0 * 8 + L // 16], L, L, 128,
                            queue_num=gq % 4)
                        gq += 1
                        oh = ohp.tile([128, MAXC * 128], FP8, tag="oh")
                        oh3 = oh[:].rearrange("p (b f) -> p b f", f=128)
                        nc.vector.tensor_tensor(
                            oh3[:, :1, :],
                            doff_sb[:, col0:col0 + 1].to_broadcast(
                                [128, 1, 128]),
                            iota3.to_broadcast([128, 1, 128]),
                            ALU.is_equal)
                        for j in range(nb):
                            w, first, last, segs = plan.blocks[b0 + j]
                            if first:
                                psums[w] = winp.tile([128, F], F32, tag="win",
                                                     name=f"w_s{s}_h{h}_{w}")
                            nseg = len(segs)
                            for si, (p, colid) in enumerate(segs):
                                nc.tensor.matmul(
                                    psums[w][:],
                                    lhsT=oh3[:, 0, :],
                                    rhs=msg[:, j, p * 64:(p + 1) * 64],
                                    start=(first and si == 0),
                                    stop=(last and si == nseg - 1))
                            if last:
                                ps = psums.pop(w)
                                if h == 0:
                                    # stash A-pass partial
                                    nc.vector.tensor_copy(st3[:, w, :], ps[:])
                                else:
                                    tmp = tmpp.tile([128, F], F32, tag="tmp")
                                    nc.vector.tensor_add(tmp[:], ps[:],
                                                         st3[:, w, :])
                                    if s < n_steps:
                                        nc.scalar.activation(
                                            y3[:, w, :], tmp[:], AF.Copy,
                                            scale=dinv2_t[:, w:w + 1])
                                    if s % 2 == 0:
                                        m = s // 2
                                        tw = twp.tile([128, F], F32, tag="tw")
                                        nc.scalar.activation(
                                            tw[:], tmp[:], AF.Copy,
                                            scale=tdinv_t[:, (m - 1) * NT + w:
                                                          (m - 1) * NT + w + 1])
                                        nc.vector.tensor_add(
                                            hid3[:, w, :], hid3[:, w, :], tw[:])
                        pass
                    # AG disabled for timing probe

                # ---- log_softmax ----
                with tc.tile_pool(name="soft", bufs=1) as softp:
                    CO = cfg.cout
                    hsl = hid3[:, :, :CO]
                    mx = softp.tile([128, NT], F32, tag="mx")
                    nc.vector.tensor_reduce(mx[:], hsl, mybir.AxisListType.X,
                                            ALU.max)
                    ex = softp.tile([128, NT * F], F32, tag="scratch")
                    ex3 = ex[:].rearrange("p (t f) -> p t f", f=F)[:, :, :CO]
                    nc.vector.tensor_tensor(
                        ex3, hsl, mx[:].to_broadcast([128, NT, CO]),
                        ALU.subtract)
                    nc.scalar.activation(ex3, ex3, AF.Exp)
                    sm = softp.tile([128, NT], F32, tag="sm")
                    nc.vector.tensor_reduce(sm[:], ex3, mybir.AxisListType.X,
                                            ALU.add)
                    ln = softp.tile([128, NT], F32, tag="ln")
                    nc.scalar.activation(ln[:], sm[:], AF.Ln)
                    ml = softp.tile([128, NT], F32, tag="ml")
                    nc.vector.tensor_add(ml[:], mx[:], ln[:])
                    ot = softp.tile([128, NT * CO], F32, tag="ot")
                    ot3 = ot[:].rearrange("p (t f) -> p t f", f=CO)
                    nc.vector.tensor_tensor(
                        ot3, hsl, ml[:].to_broadcast([128, NT, CO]),
                        ALU.subtract)
                    out3 = out_d[:].rearrange("(t p) f -> p t f", p=128)
                    nc.sync.dma_start(out3, ot3)

    nc.compile()
    return nc


# --------------------------------------------------------------------------
# entry point
# --------------------------------------------------------------------------

def kernel_with_results(x, edge_index, W1, b1, W2, b2, temp, trace=False):
    cfg = CFG
    in_maps, plan, temps = preprocess(x, edge_index, W1, b1, W2, b2, temp, cfg)
    nc = build_program(cfg, plan, temps)
    res = run_bass_kernel_spmd(nc, in_maps, core_ids=list(range(cfg.n_cores)),
                               trace=trace)
    outs = [res.results[c]["out"] for c in range(cfg.n_cores)]
    full = np.concatenate(outs, axis=0)[: cfg.n]
    return full.astype(np.float32), res


def kernel(x, edge_index, W1, b1, W2, b2, temp):
    out, _ = kernel_with_results(x, edge_index, W1, b1, W2, b2, temp)
    return out


# revision 9
# speedup vs baseline: 1.8308x; 1.5023x over previous
"""EvenNet (even-order PPR GNN) Trainium2 kernel, 8-core SPMD.

Math: hidden = sum_{k=0..5} temp[k] * A_hat^{2k} @ MLP(x); out = log_softmax(hidden)
with A_hat = D^-1/2 A D^-1/2 (D = out-degree from src indices).

Reformulation: iterate in y-space, y = D^-1/2 x.  One propagation x <- A_hat x
becomes y <- D^-1 * (A y) where (A y)[d] = sum_{e: dst=d} y[src_e] — a pure
unweighted gather + segment-sum.  Per-node scalings fold into window flushes.

Distribution: nodes sharded contiguously across 8 cores; edges routed to the
dst-owner core.  The replicated y-table is split into two DRAM halves, each
interleaving every core's half-shard:
  tableA row c*6272 + l  = y of node c*12544 + l         (l < 6272)
  tableB row c*6272 + l' = y of node c*12544 + 6272 + l'
so pair indices stay < 25088 (int16 dma_gather limit) AND each half can be
AllGathered independently: AG_A fires mid-step (after local windows 0..48
flush), AG_B at step end.  The next step's A-pass gathers depend only on
AG_A, so the Q7 descriptor generator runs straight through the AG_B barrier.

Per step each core runs an A-pass then a B-pass over its 98 dst windows:
gather chunks (1024 edges, 4 SWDGE queues) read one table half; per 128-edge
block a one-hot [edges x 128] fp8 matrix (DVE compare vs iota from
precomputed dst offsets) scatter-sums gathered bf16 pair-rows on the
TensorEngine into the window PSUM.  Blocks mix both parities: edges are
sorted by parity inside each (window, half) group, so most blocks emit one
matmul; the straddling block emits two (one per parity, disjoint one-hot
columns).  A-pass partials stash to SBUF f32; B-pass adds them back and
applies the D^-1 / temp scalings.
"""

import dataclasses
import math
import numpy as np
import ml_dtypes

from concourse import bacc, bass, mybir, tile
from concourse.bass_utils import run_bass_kernel_spmd
from concourse.masks import make_identity

F32 = mybir.dt.float32
BF16 = mybir.dt.bfloat16
FP8 = mybir.dt.float8e4
I16 = mybir.dt.int16
AF = mybir.ActivationFunctionType
ALU = mybir.AluOpType
NPBF16 = ml_dtypes.bfloat16


@dataclasses.dataclass
class Cfg:
    n_cores: int = 8
    n: int = 100000          # real nodes
    cin: int = 500           # input channels
    hid: int = 256           # MLP hidden
    cout: int = 50           # classes
    k_half: int = 5          # outer iterations (2 props each)
    chunk_blocks: int = 8    # 128-edge blocks per dma_gather (1024 = ring cap)
    sw: int = 4              # windows per super-window (PSUM rotation)
    f: int = 64              # padded feature dim

    @property
    def nsh(self):           # nodes per core (padded)
        return int(math.ceil(self.n / self.n_cores / 128) * 128)

    @property
    def nt(self):            # windows per core
        return self.nsh // 128

    @property
    def hw_(self):           # windows in half A
        return (self.nt + 1) // 2

    @property
    def hn(self):            # nodes per half-shard
        return self.hw_ * 128

    @property
    def cpad(self):
        return int(math.ceil(self.cin / 128) * 128)

    @property
    def tab_rows(self):      # rows per table half
        return self.hn * self.n_cores

    @property
    def tab_pairs(self):
        return self.tab_rows // 2


CFG = Cfg()


@dataclasses.dataclass
class Plan:
    blocks: list     # (w, start, stop, segs) per stream block; segs=[(par, col)]
    chunks: list     # (h, b0, nb, col0, ncols)
    nblk: int
    ncols: int
    ag1_chunk: int   # chunk index after which windows 0..hw_-1 have flushed


def _wrap16(arr):
    L = arr.shape[0]
    assert L % 16 == 0
    w = np.ascontiguousarray(arr.reshape(L // 16, 16).T)
    return np.tile(w, (8, 1))


def _tile_major(v, cfg):
    return np.ascontiguousarray(v.reshape(cfg.nt, 128).T)


# --------------------------------------------------------------------------
# host-side preprocessing
# --------------------------------------------------------------------------

def preprocess(x, edge_index, W1, b1, W2, b2, temp, cfg):
    n, NT, HW, HN = cfg.n, cfg.nt, cfg.hw_, cfg.hn
    NSH = cfg.nsh
    src = np.asarray(edge_index[0]).astype(np.int64)
    dst = np.asarray(edge_index[1]).astype(np.int64)
    x = np.asarray(x, dtype=np.float32)
    W1 = np.asarray(W1, dtype=np.float32)
    W2 = np.asarray(W2, dtype=np.float32)
    b1 = np.asarray(b1, dtype=np.float32)
    b2 = np.asarray(b2, dtype=np.float32)
    temp = np.asarray(temp, dtype=np.float32)
    assert not np.any(b1) and not np.any(b2), "bias path removed"

    deg = np.bincount(src, minlength=n).astype(np.float32)
    dinv = np.where(deg > 0, 1.0 / np.sqrt(np.maximum(deg, 1e-12)), 0.0).astype(np.float32)
    dinv2 = (dinv * dinv).astype(np.float32)
    temps = [float(t) for t in temp]

    # ---- per-edge routing ----
    c_s = src // NSH
    l_s = src % NSH
    half = l_s // HN                       # which table
    row = c_s * HN + (l_s % HN)            # row within table half
    pair = row // 2
    par = row % 2

    owner = dst // NSH
    ngrp = NT * 2                          # (w, half)
    counts = np.zeros((cfg.n_cores, ngrp, 2), dtype=np.int64)
    per_core = []
    for c in range(cfg.n_cores):
        m = owner == c
        dl = dst[m] - c * NSH
        w = dl // 128
        gid = w * 2 + half[m]
        o = np.lexsort((pair[m], par[m], gid))
        per_core.append((pair[m][o], (dl % 128)[o], par[m][o], gid[o]))
        cnt2 = np.zeros((ngrp, 2), dtype=np.int64)
        np.add.at(cnt2, (gid[o], par[m][o]), 1)
        counts[c] = cnt2

    tot_cnt = counts.sum(axis=2)           # [cores, ngrp]
    nblocks = np.maximum(np.ceil(tot_cnt.max(axis=0) / 128).astype(np.int64), 1)

    # ---- shared block stream: A-pass then B-pass, sw-rotated windows ----
    blocks = []            # (w, gid) per stream slot
    chunk_bounds = []      # (h, start_block, nb) chunks
    for h in (0, 1):
        pass_start = len(blocks)
        for W0 in range(0, NT, cfg.sw):
            per_w = []
            for w in range(W0, min(W0 + cfg.sw, NT)):
                per_w.append([(w, w * 2 + h)] * int(nblocks[w * 2 + h]))
            while any(per_w):
                for lst in per_w:
                    if lst:
                        blocks.append(lst.pop(0))
        for i in range(pass_start, len(blocks), cfg.chunk_blocks):
            nb = min(cfg.chunk_blocks, len(blocks) - i)
            chunk_bounds.append((h, i, nb))
    nblk = len(blocks)

    # first/last per (w, h) pass segment
    first_seen, last_idx = {}, {}
    for i, (w, gid) in enumerate(blocks):
        if gid not in first_seen:
            first_seen[gid] = i
        last_idx[gid] = i

    # ordered stream positions of each group's blocks (interleaved by the
    # super-window round-robin)
    gblocks = {}
    for i, (w, gid) in enumerate(blocks):
        gblocks.setdefault(gid, []).append(i)

    # ---- segments (parity runs) per block ----
    # Per core, group gid's slots: par0 edges at [0, cnt0), par1 at
    # [cnt0, cnt0+cnt1).  Shared flags OR over cores.
    cnt0 = counts[:, :, 0]                 # per core
    cnt1 = counts[:, :, 1]
    has = np.zeros((nblk, 2), dtype=bool)
    for g in range(ngrp):
        if g not in gblocks:
            continue
        bl = gblocks[g]
        for k, bi in enumerate(bl):
            lo, hi = k * 128, (k + 1) * 128
            for c in range(cfg.n_cores):
                c0 = int(cnt0[c, g])
                ctot = c0 + int(cnt1[c, g])
                if min(c0, hi) > lo:
                    has[bi, 0] = True
                if min(ctot, hi) > max(c0, lo):
                    has[bi, 1] = True
        if not has[bl].any():
            has[bl[0], 0] = True           # empty group: dummy even segment

    # column assignment (one one-hot column per (block, parity) segment)
    segs_per_block = []
    col = 0
    for i in range(nblk):
        segs = [(p, None) for p in (0, 1) if has[i, p]]
        segs = [(p, col + j) for j, (p, _) in enumerate(segs)]
        col += len(segs)
        segs_per_block.append(segs)
    ncols = col

    blocks_meta = []
    for i, (w, gid) in enumerate(blocks):
        blocks_meta.append((w, i == first_seen[gid], i == last_idx[gid],
                            segs_per_block[i]))

    # chunks with column ranges
    chunks = []
    for (h, b0, nb) in chunk_bounds:
        col0 = segs_per_block[b0][0][1]
        last = segs_per_block[b0 + nb - 1]
        colend = last[-1][1] + 1
        chunks.append((h, b0, nb, col0, colend - col0))

    # AG1 point: after the chunk containing the stop-B block of window HW-1
    wtarget = HW - 1
    stopB_blk = last_idx[wtarget * 2 + 1]
    ag1_chunk = next(ci for ci, (h, b0, nb, _, _) in enumerate(chunks)
                     if h == 1 and b0 <= stopB_blk < b0 + nb)

    # ---- per-core index / dstoff arrays ----
    tot = nblk * 128
    gidx_maps, doff_maps = [], []
    for c in range(cfg.n_cores):
        pr, dl, pa, gid = per_core[c]
        g_arr = (np.arange(tot, dtype=np.int64) % cfg.tab_pairs).astype(np.int16)
        d_arr = np.full((ncols, 128), -1.0, dtype=np.float32)
        gb = np.concatenate([[0], np.cumsum(tot_cnt[c])])
        for g in range(ngrp):
            cnt = int(tot_cnt[c, g])
            if cnt == 0:
                continue
            bl = np.asarray(gblocks[g], dtype=np.int64)
            sl = slice(gb[g], gb[g + 1])
            j = np.arange(cnt)
            blk = bl[j // 128]
            slot = j % 128
            pos = blk * 128 + slot
            g_arr[pos] = pr[sl].astype(np.int16)
            # dstoff goes to the column matching (block, parity)
            par_e = pa[sl]
            for b in np.unique(blk):
                mb = blk == b
                for (p, colid) in segs_per_block[b]:
                    mm = mb & (par_e == p)
                    d_arr[colid, slot[mm]] = dl[sl][mm]
        gidx_maps.append(_wrap16(g_arr))
        doff_maps.append(np.ascontiguousarray(d_arr.T).astype(NPBF16))

    # ---- dense per-core inputs (bf16 MLP) ----
    W1p = np.zeros((cfg.cpad, cfg.hid), dtype=NPBF16)
    W1p[: cfg.cin] = W1.astype(NPBF16)
    W2p = np.zeros((cfg.hid, cfg.f), dtype=NPBF16)
    W2p[:, : cfg.cout] = W2.astype(NPBF16)
    iota = np.tile(np.arange(128, dtype=np.float32)[None, :], (128, 1)).astype(NPBF16)
    in_maps = []
    for c in range(cfg.n_cores):
        lo = c * NSH
        real = max(0, min(NSH, n - lo))
        xs = np.zeros((NSH, cfg.cpad), dtype=NPBF16)
        if real > 0:
            xs[:real, : cfg.cin] = x[lo:lo + real].astype(NPBF16)
        dv = np.zeros(NSH, np.float32)
        dv[:real] = dinv[lo:lo + real]
        dv2 = np.zeros(NSH, np.float32)
        dv2[:real] = dinv2[lo:lo + real]
        tdv = np.zeros((128, cfg.k_half * NT), np.float32)
        for m in range(1, cfg.k_half + 1):
            tdv[:, (m - 1) * NT:m * NT] = _tile_major(
                (temps[m] * dv).astype(np.float32), cfg)
        in_maps.append({
            "xs": xs,
            "w1": W1p,
            "w2": W2p,
            "gidx": gidx_maps[c],
            "dstoff": doff_maps[c],
            "iota": iota,
            "dinv_t": _tile_major(dv, cfg),
            "dinv2_t": _tile_major(dv2, cfg),
            "tdinv": tdv,
        })

    plan = Plan(blocks=blocks_meta, chunks=chunks, nblk=nblk, ncols=ncols,
                ag1_chunk=ag1_chunk)
    return in_maps, plan, temps


# --------------------------------------------------------------------------
# program builder
# --------------------------------------------------------------------------

def build_program(cfg, plan, temps):
    nc = bacc.Bacc("TRN2", target_bir_lowering=False, debug=False,
                   num_devices=cfg.n_cores, num_swdge_queues=4)

    NT, F, NSH, HW, HN = cfg.nt, cfg.f, cfg.nsh, cfg.hw_, cfg.hn
    NCH, NHC = cfg.cpad // 128, cfg.hid // 128
    nblk, ncols = plan.nblk, plan.ncols
    CB = cfg.chunk_blocks
    MAXC = 2 * CB

    xs_d = nc.declare_dram_parameter("xs", [NSH, cfg.cpad], BF16, isOutput=False)
    w1_d = nc.declare_dram_parameter("w1", [cfg.cpad, cfg.hid], BF16, isOutput=False)
    w2_d = nc.declare_dram_parameter("w2", [cfg.hid, F], BF16, isOutput=False)
    gidx_d = nc.declare_dram_parameter("gidx", [128, nblk * 8], I16, isOutput=False)
    doff_d = nc.declare_dram_parameter("dstoff", [128, ncols], BF16, isOutput=False)
    iota_d = nc.declare_dram_parameter("iota", [128, 128], BF16, isOutput=False)
    dinv_d = nc.declare_dram_parameter("dinv_t", [128, NT], F32, isOutput=False)
    dinv2_d = nc.declare_dram_parameter("dinv2_t", [128, NT], F32, isOutput=False)
    tdinv_d = nc.declare_dram_parameter("tdinv", [128, cfg.k_half * NT], F32,
                                        isOutput=False)
    out_d = nc.declare_dram_parameter("out", [NSH, cfg.cout], F32, isOutput=True)

    tabA = nc.dram_tensor("ytabA", [cfg.tab_rows, F], BF16, addr_space="Shared")
    tabB = nc.dram_tensor("ytabB", [cfg.tab_rows, F], BF16, addr_space="Shared")
    bounceA = nc.dram_tensor("ybounceA", [HN, F], BF16)
    bounceB = nc.dram_tensor("ybounceB", [HN, F], BF16)
    tabA_v = tabA[:].rearrange("(a b) f -> a (b f)", b=2)   # [pairs, 128]
    tabB_v = tabB[:].rearrange("(a b) f -> a (b f)", b=2)

    n_steps = 2 * cfg.k_half
    rg = [list(range(cfg.n_cores))]

    with tile.TileContext(nc) as tc:
        with (
            tc.tile_pool(name="const", bufs=1) as constp,
            tc.tile_pool(name="persist", bufs=1) as persist,
        ):
            w1sb = constp.tile([128, NCH * cfg.hid], BF16)
            for c in range(NCH):
                nc.sync.dma_start(w1sb[:, c * cfg.hid:(c + 1) * cfg.hid],
                                  w1_d[c * 128:(c + 1) * 128, :])
            w2sb = constp.tile([128, NHC * F], BF16)
            for c in range(NHC):
                nc.sync.dma_start(w2sb[:, c * F:(c + 1) * F],
                                  w2_d[c * 128:(c + 1) * 128, :])
            dinv_t = constp.tile([128, NT], F32)
            nc.sync.dma_start(dinv_t[:], dinv_d[:])
            dinv2_t = constp.tile([128, NT], F32)
            nc.sync.dma_start(dinv2_t[:], dinv2_d[:])
            tdinv_t = constp.tile([128, cfg.k_half * NT], F32)
            nc.sync.dma_start(tdinv_t[:], tdinv_d[:])
            doff_sb = constp.tile([128, ncols], BF16)
            nc.sync.dma_start(doff_sb[:], doff_d[:])
            gidx_sb = constp.tile([128, nblk * 8], I16)
            nc.sync.dma_start(gidx_sb[:], gidx_d[:])
            iota_sb = constp.tile([128, 128], BF16)
            nc.sync.dma_start(iota_sb[:], iota_d[:])
            iota3 = iota_sb[:].rearrange("p (a f) -> p a f", a=1)

            hid_sb = persist.tile([128, NT * F], F32)
            ysb = persist.tile([128, NT * F], BF16)
            stash = persist.tile([128, NT * F], F32)
            hid3 = hid_sb[:].rearrange("p (t f) -> p t f", f=F)
            y3 = ysb[:].rearrange("p (t f) -> p t f", f=F)
            st3 = stash[:].rearrange("p (t f) -> p t f", f=F)

            bounceA3 = bounceA[:].rearrange("(t p) f -> p t f", p=128)
            bounceB3 = bounceB[:].rearrange("(t p) f -> p t f", p=128)

            def ag_a():
                nc.sync.dma_start(bounceA3, y3[:, :HW, :])
                nc.gpsimd.collective_compute(
                    "AllGather", ALU.bypass, replica_groups=rg,
                    ins=[bounceA[:]], outs=[tabA[:]])

            def ag_b():
                nc.sync.dma_start(bounceB3, y3[:, HW:, :])
                nc.gpsimd.collective_compute(
                    "AllGather", ALU.bypass, replica_groups=rg,
                    ins=[bounceB[:]], outs=[tabB[:]])

            # ---- MLP (bf16) ----
            with (
                tc.tile_pool(name="xload", bufs=3) as xload,
                tc.tile_pool(name="xT", bufs=8) as xTp,
                tc.tile_pool(name="h1", bufs=2) as h1p,
                tc.tile_pool(name="h1T", bufs=4) as h1Tp,
                tc.tile_pool(name="psT", bufs=4, space="PSUM") as psT,
                tc.tile_pool(name="psH", bufs=2, space="PSUM") as psH,
                tc.tile_pool(name="psO", bufs=2, space="PSUM") as psO,
            ):
                ident = constp.tile([128, 128], BF16)
                make_identity(nc, ident[:])
                for t in range(NT):
                    xt = xload.tile([128, cfg.cpad], BF16)
                    nc.sync.dma_start(xt[:], xs_d[t * 128:(t + 1) * 128, :])
                    xTs = []
                    for c in range(NCH):
                        pt = psT.tile([128, 128], BF16)
                        nc.tensor.transpose(pt[:], xt[:, c * 128:(c + 1) * 128],
                                            ident[:])
                        xTc = xTp.tile([128, 128], BF16)
                        if c % 2 == 0:
                            nc.vector.tensor_copy(xTc[:], pt[:])
                        else:
                            nc.scalar.mul(xTc[:], pt[:], 1.0)
                        xTs.append(xTc)
                    h1ps = psH.tile([128, cfg.hid], F32)
                    for c in range(NCH):
                        nc.tensor.matmul(h1ps[:], lhsT=xTs[c][:],
                                         rhs=w1sb[:, c * cfg.hid:(c + 1) * cfg.hid],
                                         start=(c == 0), stop=(c == NCH - 1))
                    h1 = h1p.tile([128, cfg.hid], BF16)
                    nc.scalar.activation(h1[:], h1ps[:], AF.Relu)
                    h1Ts = []
                    for c in range(NHC):
                        pt = psT.tile([128, 128], BF16)
                        nc.tensor.transpose(pt[:], h1[:, c * 128:(c + 1) * 128],
                                            ident[:])
                        hTc = h1Tp.tile([128, 128], BF16)
                        if c % 2 == 0:
                            nc.vector.tensor_copy(hTc[:], pt[:])
                        else:
                            nc.scalar.mul(hTc[:], pt[:], 1.0)
                        h1Ts.append(hTc)
                    hps = psO.tile([128, F], F32)
                    for c in range(NHC):
                        nc.tensor.matmul(hps[:], lhsT=h1Ts[c][:],
                                         rhs=w2sb[:, c * F:(c + 1) * F],
                                         start=(c == 0), stop=(c == NHC - 1))
                    nc.scalar.mul(hid_sb[:, t * F:(t + 1) * F], hps[:], temps[0])
                    nc.vector.tensor_scalar_mul(ysb[:, t * F:(t + 1) * F],
                                                hps[:], dinv_t[:, t:t + 1])
                    if t == HW - 1:
                        ag_a()
                ag_b()

            # ---- propagation steps ----
            with (
                tc.tile_pool(name="msg", bufs=12) as msgp,
                tc.tile_pool(name="oh", bufs=8) as ohp,
                tc.tile_pool(name="tmp", bufs=4) as tmpp,
                tc.tile_pool(name="tw", bufs=4) as twp,
                tc.tile_pool(name="win", bufs=2 * cfg.sw, space="PSUM") as winp,
            ):
                gq = 0
                for s in range(1, n_steps + 1):
                    psums = {}
                    for ci, (h, b0, nb, col0, nco) in enumerate(plan.chunks):
                        L = nb * 128
                        msg = msgp.tile([128, CB, 128], BF16, tag="msg")
                        tbl = tabA_v if h == 0 else tabB_v
                        nc.gpsimd.dma_gather(
                            msg[:, :nb, :], tbl,
                            gidx_sb[:, b0 * 8:b0 * 8 + L // 16], L, L, 128,
                            queue_num=gq % 4)
                        gq += 1
                        oh = ohp.tile([128, MAXC * 128], FP8, tag="oh")
                        oh3 = oh[:].rearrange("p (b f) -> p b f", f=128)
                        nc.vector.tensor_tensor(
                            oh3[:, :nco, :],
                            doff_sb[:, col0:col0 + nco].to_broadcast(
                                [128, nco, 128]),
                            iota3.to_broadcast([128, nco, 128]),
                            ALU.is_equal)
                        for j in range(nb):
                            w, first, last, segs = plan.blocks[b0 + j]
                            if first:
                                psums[w] = winp.tile([128, F], F32, tag="win",
                                                     name=f"w_s{s}_h{h}_{w}")
                            nseg = len(segs)
                            for si, (p, colid) in enumerate(segs):
                                nc.tensor.matmul(
                                    psums[w][:],
                                    lhsT=oh3[:, colid - col0, :],
                                    rhs=msg[:, j, p * 64:(p + 1) * 64],
                                    start=(first and si == 0),
                                    stop=(last and si == nseg - 1))
                            if last:
                                ps = psums.pop(w)
                                if h == 0:
                                    # stash A-pass partial
                                    nc.vector.tensor_copy(st3[:, w, :], ps[:])
                                else:
                                    tmp = tmpp.tile([128, F], F32, tag="tmp")
                                    nc.vector.tensor_add(tmp[:], ps[:],
                                                         st3[:, w, :])
                                    if s < n_steps:
                                        nc.scalar.activation(
                                            y3[:, w, :], tmp[:], AF.Copy,
                                            scale=dinv2_t[:, w:w + 1])
                                    if s % 2 == 0:
                                        m = s // 2
                                        tw = twp.tile([128, F], F32, tag="tw")
                                        nc.scalar.activation(
                                            tw[:], tmp[:], AF.Copy,
                                            scale=tdinv_t[:, (m - 1) * NT + w:
                                                          (m - 1) * NT + w + 1])
                                        nc.vector.tensor_add(
                                            hid3[:, w, :], hid3[:, w, :], tw[:])
                        if ci == plan.ag1_chunk and s < n_steps:
                            ag_a()
                    if s < n_steps:
                        ag_b()

                # ---- log_softmax ----
                with tc.tile_pool(name="soft", bufs=1) as softp:
                    CO = cfg.cout
                    hsl = hid3[:, :, :CO]
                    mx = softp.tile([128, NT], F32, tag="mx")
                    nc.vector.tensor_reduce(mx[:], hsl, mybir.AxisListType.X,
                                            ALU.max)
                    ex = softp.tile([128, NT * F], F32, tag="scratch")
                    ex3 = ex[:].rearrange("p (t f) -> p t f", f=F)[:, :, :CO]
                    nc.vector.tensor_tensor(
                        ex3, hsl, mx[:].to_broadcast([128, NT, CO]),
                        ALU.subtract)
                    nc.scalar.activation(ex3, ex3, AF.Exp)
                    sm = softp.tile([128, NT], F32, tag="sm")
                    nc.vector.tensor_reduce(sm[:], ex3, mybir.AxisListType.X,
                                            ALU.add)
                    ln = softp.tile([128, NT], F32, tag="ln")
                    nc.scalar.activation(ln[:], sm[:], AF.Ln)
                    ml = softp.tile([128, NT], F32, tag="ml")
                    nc.vector.tensor_add(ml[:], mx[:], ln[:])
                    ot = softp.tile([128, NT * CO], F32, tag="ot")
                    ot3 = ot[:].rearrange("p (t f) -> p t f", f=CO)
                    nc.vector.tensor_tensor(
                        ot3, hsl, ml[:].to_broadcast([128, NT, CO]),
                        ALU.subtract)
                    out3 = out_d[:].rearrange("(t p) f -> p t f", p=128)
                    nc.sync.dma_start(out3, ot3)

    nc.compile()
    return nc


# --------------------------------------------------------------------------
# entry point
# --------------------------------------------------------------------------

def kernel_with_results(x, edge_index, W1, b1, W2, b2, temp, trace=False):
    cfg = CFG
    in_maps, plan, temps = preprocess(x, edge_index, W1, b1, W2, b2, temp, cfg)
    nc = build_program(cfg, plan, temps)
    res = run_bass_kernel_spmd(nc, in_maps, core_ids=list(range(cfg.n_cores)),
                               trace=trace)
    outs = [res.results[c]["out"] for c in range(cfg.n_cores)]
    full = np.concatenate(outs, axis=0)[: cfg.n]
    return full.astype(np.float32), res


def kernel(x, edge_index, W1, b1, W2, b2, temp):
    out, _ = kernel_with_results(x, edge_index, W1, b1, W2, b2, temp)
    return out
